# revision 3
# baseline (speedup 1.0000x reference)
"""Trainium2 Bass kernel for BDH recurrent (chunked linear) attention.

Problem shapes (hardcoded): Q_raw [2,16,2048,256] f32, V_raw [2,2048,1024] f32,
out [2,16,2048,1024] f32.  8 NeuronCores, data+head parallel: each core owns
4 (batch, head) pairs; V is shared across the 4 heads of a core's batch.

Math (reference semantics), per (b,h), chunks of 128:
  QR = rope(Q); KR = QR
  out_c = q_c @ state_{<c} + (q_c q_c^T  * strict_tril) v_c
  state += q_c^T v_c

Design:
  * RoPE is a fixed elementwise map of the input, so it is precomputed on
    the host (in fp32, then cast fp16) in the layouts the matmuls need.
  * Per (bh, superchunk) the transposed q (G lhsT/rhs + inter lhsT) and
    natural q (state-update lhsT) are packed in ONE 256KB DRAM piece with
    2KB-contiguous per-partition lines (1 descriptor per DMA engine), so
    each piece moves at wire speed with minimal latency; the first piece
    gates the first matmul at ~9.5us instead of 12.5us.
  * fp16 for all 16-bit work; fp8(e4m3) DoubleRow PV matmuls: per
    superchunk of SUP=2 chunks, the transposed score blocks G_j are
    evacuated into one [128, 2, 256] fp8 pair tile; the PV for chunk i is
    ONE DoubleRow matmul per D-half contracting 256 rows at 2x rate.
    The pair row of the later chunk j1 is zeroed in its leading 128 cols
    by a gpsimd memset (free engine), the rest by mask-multiplies on DVE.
  * PSUM-resident fp32 state; cast to fp16 SBUF right after each
    superchunk's accumulation group closes, split by m-plane across
    scalar (m0) and vector (m1) so the next superchunk's inter matmuls
    unblock as early as possible.
  * PSUM out evacuation split across scalar/vector by a 5:3 chunk
    pattern to balance the two PSUM-capable engines.
All DRAM layouts are partition-major; the output is written
partition-major and un-permuted on host.
"""

import numpy as np
import ml_dtypes

import concourse.mybir as mybir
import concourse.tile as tile
from concourse import bacc
from concourse.bass import ds
from concourse.bass_utils import run_bass_kernel_spmd

B, NH, T, N, D = 2, 16, 2048, 256, 1024
P = 128          # partition / chunk size
NCH = T // P     # 16 chunks
SUP = 2          # chunks per superchunk
NSUP = NCH // SUP
HPC = 4          # (b,h) pairs per core
NCORES = 8
THETA = 2.0 ** 16
TWO_PI = 2.0 * np.pi

f16 = mybir.dt.float16
f8 = mybir.dt.float8e4
f32 = mybir.dt.float32
f16_np = np.float16
f8_np = ml_dtypes.float8_e4m3  # TRN-style e4m3 (max normal 240)

mult = mybir.AluOpType.mult
DR = mybir.MatmulPerfMode.DoubleRow

# engine assignment knobs.
EVAC8 = ("s", "s", "v", "s", "v", "s", "v", "s")  # per chunk (i % 8)
STATE_CAST_ENG = ("s", "v")   # by m-plane


def _copy(nc, c, out, in_):
    if c == "s":
        nc.scalar.copy(out, in_)
    else:
        nc.vector.tensor_copy(out, in_)


def _emit_body(nc, tc, qX, v, v8, mskT, out):
    """Tile program for one core: 4 (b,h) pairs, full scan each."""
    with (
        tc.tile_pool(name="const", bufs=1) as constp,
        tc.tile_pool(name="qpool", bufs=2) as qpool,
        tc.tile_pool(name="work", bufs=4) as work,
        tc.tile_pool(name="outbuf", bufs=2) as outp,
        tc.tile_pool(name="statesb", bufs=2) as statep,
        tc.tile_pool(name="ps_state", bufs=1, space="PSUM") as ps_state,
        tc.tile_pool(name="ps_out", bufs=3, space="PSUM") as ps_out,
        tc.tile_pool(name="ps_g", bufs=1, space="PSUM") as ps_g,
    ):
        msk_sb = constp.tile([P, 384], f16)
        v8_sb = constp.tile([P, NCH, D], f8)
        v_sb = constp.tile([P, NCH, D], f16)
        # Early pieces ordered by first consumption.  sync carries the
        # critical path (mask -> v8[0:2] -> v[0] -> v[1]); scalar (whose
        # first issue slot is delayed ~1.3us by its ACT table load) takes
        # mid-scan pieces with large slack.
        nc.sync.dma_start(msk_sb[:], mskT[:, :])
        nc.sync.dma_start(v8_sb[:, 0:2], v8[:, 0:2, :])
        nc.sync.dma_start(v_sb[:, 0:1], v[:, 0:1, :])
        nc.sync.dma_start(v_sb[:, 1:2], v[:, 1:2, :])
        nc.sync.dma_start(v8_sb[:, 2:4], v8[:, 2:4, :])
        nc.sync.dma_start(v_sb[:, 2:4], v[:, 2:4, :])
        nc.scalar.dma_start(v8_sb[:, 4:8], v8[:, 4:8, :])
        nc.scalar.dma_start(v_sb[:, 4:8], v[:, 4:8, :])
        nc.sync.dma_start(v8_sb[:, 8:12], v8[:, 8:12, :])
        nc.sync.dma_start(v_sb[:, 8:12], v[:, 8:12, :])
        nc.scalar.dma_start(v8_sb[:, 12:16], v8[:, 12:16, :])
        nc.scalar.dma_start(v_sb[:, 12:16], v[:, 12:16, :])

        def bh_prologue(bh):
            """Allocate the per-bh q tile + emit per-superchunk DMAs."""
            q_sb = qpool.tile([P, NSUP, 1024], f16, tag="q", name=f"q{bh}")
            if bh == 0:
                # cold start: split sup0's piece so the qt half (cols
                # 0:512, feeds G) lands first
                nc.gpsimd.dma_start(q_sb[:, 0, ds(0, 512)], qX[bh, 0, :, ds(0, 512)])
                nc.gpsimd.dma_start(q_sb[:, 0, ds(512, 512)], qX[bh, 0, :, ds(512, 512)])
                start = 1
            else:
                start = 0
            for s in range(start, NSUP):
                nc.gpsimd.dma_start(q_sb[:, s, :], qX[bh, s, :, :])
            return q_sb

        # within a sup piece: qt(m, t') at col m*256 + t'; qn(m, ci, k)
        # at col 512 + m*256 + ci*128 + k.
        def emit_G(q_tile, s):
            # Transposed score blocks for superchunk s's two chunks into
            # one PSUM tile: G_j0 at cols 0:256 (diag block + the j1
            # block), G_j1 at 256:384; then the masked fp8 evacuation
            # into a pair tile [p, j', i-col]: row 0 = G_j0 (diag-masked
            # then ones), row 1 = [zeros (gpsimd memset) | G_j1
            # diag-masked].  Called one superchunk AHEAD so the
            # evacuation is off the boundary's critical path.
            g_ps = ps_g.tile([P, 384], f32, tag="g", name="g_ps")
            nc.tensor.matmul(
                g_ps[:, 0:256], q_tile[:, s, ds(0, 128)],
                q_tile[:, s, ds(0, 256)], start=True, stop=False,
            )
            nc.tensor.matmul(
                g_ps[:, 0:256], q_tile[:, s, ds(256, 128)],
                q_tile[:, s, ds(256, 256)], start=False, stop=True,
            )
            nc.tensor.matmul(
                g_ps[:, 256:384], q_tile[:, s, ds(128, 128)],
                q_tile[:, s, ds(128, 128)], start=True, stop=False,
            )
            nc.tensor.matmul(
                g_ps[:, 256:384], q_tile[:, s, ds(384, 128)],
                q_tile[:, s, ds(384, 128)], start=False, stop=True,
            )
            g2 = work.tile([P, 2, 256], f8, tag="g2", name="g2")
            nc.gpsimd.memset(g2[:, 1, ds(0, 128)], 0.0)
            nc.vector.tensor_tensor(g2[:, 0], g_ps[:, 0:256], msk_sb[:, ds(0, 256)], mult)
            nc.vector.tensor_tensor(
                g2[:, 1, ds(128, 128)], g_ps[:, 256:384], msk_sb[:, ds(256, 128)], mult
            )
            return g2

        nxt = bh_prologue(0)
        g2_cur = emit_G(nxt, 0)
        for bh in range(HPC):
            q_sb = nxt
            nxt = None

            state_ps = ps_state.tile([P, 2, D], f32, tag="state")
            out_sbs = [
                outp.tile([P, NCH // 2, D], f16, tag=f"out{h}", name=f"out_sb{h}")
                for h in range(2)
            ]

            def emit_state_chunk(s, ci, i, is_start, is_close):
                # state += qr_c^T v_c (PSUM accumulate).  Each
                # superchunk's accumulation is a CLOSED group; on close,
                # each m-plane is cast fp32->fp16 on its own engine
                # (scalar m0 / vector m1) immediately, so the next
                # superchunk's inter matmuls unblock per-plane.
                sb = (
                    statep.tile([P, 2, D], f16, tag="state_sb", name="state_sb")
                    if is_close else None
                )
                for m in range(2):
                    for h in range(2):
                        dsl = ds(h * 512, 512)
                        nc.tensor.matmul(
                            state_ps[:, m, dsl],
                            q_sb[:, s, ds(512 + m * 256 + ci * 128, 128)],
                            v_sb[:, i, dsl],
                            start=is_start,
                            stop=is_close,
                            skip_group_check=True,
                        )
                    if is_close:
                        _copy(
                            nc, STATE_CAST_ENG[m], sb[:, m, :], state_ps[:, m, :]
                        )
                return sb

            state_sb_next = None
            for s in range(NSUP):
                state_sb = state_sb_next
                j0 = SUP * s
                g2 = g2_cur

                if bh < HPC - 1 and s == 4:
                    nxt = bh_prologue(bh + 1)

                for ci in range(SUP):
                    i = SUP * s + ci
                    if 0 < s < NSUP - 1:
                        # NB: matmul PSUM output is capped at 512 fp32
                        # (one bank) -> per-(m,h) matmuls.  State after
                        # the last superchunk is never read -> skipped.
                        sb = emit_state_chunk(s, ci, i, False, ci == SUP - 1)
                        if ci == SUP - 1:
                            state_sb_next = sb
                    out_ps = [
                        ps_out.tile([P, 512], f32, tag="outp", name=f"out_ps{h}")
                        for h in range(2)
                    ]
                    if s > 0:
                        # m-outer / h-inner: consecutive matmuls share lhsT
                        for m in range(2):
                            for h in range(2):
                                nc.tensor.matmul(
                                    out_ps[h][:],
                                    q_sb[:, s, ds(m * 256 + ci * 128, 128)],
                                    state_sb[:, m, ds(h * 512, 512)],
                                    start=(m == 0), stop=False,
                                    skip_group_check=True,
                                )
                    # PV: one fp8 DoubleRow matmul per D-half, contracting
                    # both chunks of the superchunk at 2x rate.
                    for h in range(2):
                        nc.tensor.matmul(
                            out_ps[h][:],
                            g2[:, :, ds(ci * P, P)],
                            v8_sb[:, ds(j0, SUP), ds(h * 512, 512)],
                            start=(s == 0), stop=True,
                            perf_mode=DR,
                            skip_group_check=True,
                        )

                    out_sb = out_sbs[i // (NCH // 2)]
                    if i == NCH - 1:
                        engs = ("s", "v")  # final chunk: minimize drain latency
                    else:
                        e = EVAC8[i % 8]
                        engs = (e, e)
                    for h in range(2):
                        _copy(
                            nc, engs[h],
                            out_sb[:, i % (NCH // 2), ds(h * 512, 512)],
                            out_ps[h][:],
                        )
                    if ci == 0:
                        # pipeline: emit the NEXT superchunk's G + fp8 evac
                        # now (PE runs it after this sup's remaining work;
                        # the DVE evac lands before the next PV needs it)
                        if s + 1 < NSUP:
                            g2_cur = emit_G(q_sb, s + 1)
                        elif bh < HPC - 1:
                            g2_cur = emit_G(nxt, 0)
                    if s == NSUP - 1 and bh == HPC - 1:
                        # drain tail: per-chunk, D-halves on two different
                        # queues so the final transfers ride parallel rings
                        nc.sync.dma_start(
                            out[bh, :, ds(i, 1), ds(0, 512)],
                            out_sbs[i // (NCH // 2)][:, ds(i % (NCH // 2), 1), ds(0, 512)],
                        )
                        nc.scalar.dma_start(
                            out[bh, :, ds(i, 1), ds(512, 512)],
                            out_sbs[i // (NCH // 2)][:, ds(i % (NCH // 2), 1), ds(512, 512)],
                        )
                    elif ci == SUP - 1:
                        for cc in range(SUP):
                            nc.sync.dma_start(
                                out[bh, :, ds(j0 + cc, 1), :],
                                out_sbs[j0 // (NCH // 2)][
                                    :, ds((j0 + cc) % (NCH // 2), 1)
                                ],
                            )

                if s == 0:
                    # sup0's state is deferred past its PV so the PV
                    # starts ASAP after the prologue.
                    for ci2 in range(SUP):
                        sb = emit_state_chunk(
                            0, ci2, ci2, ci2 == 0, ci2 == SUP - 1
                        )
                        if ci2 == SUP - 1:
                            state_sb_next = sb


_BUILT = {}


def _build():
    if "nc" in _BUILT:
        return _BUILT["nc"]
    nc = bacc.Bacc(
        "TRN2", target_bir_lowering=False, debug=False,
        enable_asserts=True, num_devices=NCORES,
    )
    qX = nc.dram_tensor("q", [HPC, NSUP, P, 1024], f16, kind="ExternalInput")
    v = nc.dram_tensor("v", [P, NCH, D], f16, kind="ExternalInput")
    v8 = nc.dram_tensor("v8", [P, NCH, D], f8, kind="ExternalInput")
    mskT = nc.dram_tensor("mskT", [P, 384], f16, kind="ExternalInput")
    out = nc.dram_tensor("out", [HPC, P, NCH, D], f16, kind="ExternalOutput")
    with tile.TileContext(nc) as tc:
        _emit_body(nc, tc, qX, v, v8, mskT, out)
    nc.compile()
    _BUILT["nc"] = nc
    return nc


def _host_prep(Q_raw, V_raw):
    """Shard + precompute device inputs (fp16/fp8, partition-major),
    including the RoPE rotation (an input-only elementwise transform),
    computed in fp32 exactly as the reference does."""
    Q = np.asarray(Q_raw, dtype=np.float32)
    V = np.asarray(V_raw, dtype=np.float32)

    t = np.arange(N, dtype=np.float32)
    q = np.floor(t / 2.0) * 2.0
    freqs = (1.0 / (THETA ** (q / np.float32(N))) / np.float32(TWO_PI)).astype(
        np.float32
    )
    phases = np.arange(T, dtype=np.float32)[:, None] * freqs[None, :]
    ph = (phases % 1.0) * np.float32(TWO_PI)
    cosf = np.cos(ph).astype(np.float32)  # [T, N]
    sinf = np.sin(ph).astype(np.float32)
    QR = np.empty_like(Q)
    Qe, Qo = Q[..., 0::2], Q[..., 1::2]
    ce, se = cosf[:, 0::2], sinf[:, 0::2]
    QR[..., 0::2] = Qe * ce - Qo * se
    QR[..., 1::2] = Qo * ce + Qe * se

    # masks [P, 384]: [0:128] strict-triu (G_j0 diag), [128:256] ones
    # (j0->j1 cross block), [256:384] strict-triu (G_j1 diag)
    mskT = np.zeros((P, 384), np.float32)
    mskT[:, 0:128] = np.triu(np.ones((P, P), np.float32), k=1)
    mskT[:, 128:256] = 1.0
    mskT[:, 256:384] = np.triu(np.ones((P, P), np.float32), k=1)
    mskT = mskT.astype(f16_np)

    # deinterleave pairs: planes (evens, odds), cast fp16
    Qd = np.stack([QR[..., 0::2], QR[..., 1::2]], axis=2).astype(f16_np)
    # Qd: [B, NH, 2, T, 128]
    # per-(bh, sup) piece, 2KB contiguous per partition:
    #   cols [0:512]    qt: [m, t'] -> Qd[b,h,m, s*256 + t', k=p]
    #   cols [512:1024] qn: [m, ci, k] -> Qd[b,h,m, s*256+ci*128+p, k]
    Qt = (
        Qd.transpose(0, 1, 4, 2, 3)              # [B,NH,k,2,T]
        .reshape(B, NH, P, 2, NSUP, SUP * P)
        .transpose(0, 1, 4, 2, 3, 5)             # [B,NH,NSUP,k,2,256]
        .reshape(B, NH, NSUP, P, 512)
    )
    Qn = (
        Qd.reshape(B, NH, 2, NSUP, SUP, P, P)    # [B,NH,m,s,ci,t',k]
        .transpose(0, 1, 3, 5, 2, 4, 6)          # [B,NH,s,t',m,ci,k]
        .reshape(B, NH, NSUP, P, 512)
    )
    q2 = np.ascontiguousarray(np.concatenate([Qt, Qn], axis=-1))
    # q2: [B, NH, NSUP, P, 1024]

    V16 = V.astype(f16_np)
    # v layout [P, NCH, D]: (p, c, d) = V[c*128+p, d]
    Vp = np.ascontiguousarray(V16.reshape(B, NCH, P, D).transpose(0, 2, 1, 3))
    V8p = Vp.astype(f8_np)

    in_maps = []
    for core in range(NCORES):
        b = core // (NCORES // B)
        hs = (core % (NCORES // B)) * HPC
        in_maps.append(
            {
                "q": np.ascontiguousarray(q2[b, hs : hs + HPC]),
                "v": Vp[b],
                "v8": V8p[b],
                "mskT": mskT,
            }
        )
    return in_maps


def _run(inputs, trace=False, **kw):
    nc = _build()
    in_maps = _host_prep(inputs["Q_raw"], inputs["V_raw"])
    res = run_bass_kernel_spmd(nc, in_maps, list(range(NCORES)), trace=trace, **kw)
    out = np.empty((B, NH, T, D), dtype=np.float32)
    for core in range(NCORES):
        b = core // (NCORES // B)
        hs = (core % (NCORES // B)) * HPC
        # device out: [HPC, P, NCH, D] partition-major -> [HPC, T, D]
        o = res.results[core]["out"].astype(np.float32)
        out[b, hs : hs + HPC] = o.transpose(0, 2, 1, 3).reshape(HPC, T, D)
    return out, res


def kernel(**inputs):
    out, _ = _run(inputs)
    return out


# revision 11
# speedup vs baseline: 1.0457x; 1.0457x over previous
"""Trainium2 Bass kernel for BDH recurrent (chunked linear) attention.

Problem shapes (hardcoded): Q_raw [2,16,2048,256] f32, V_raw [2,2048,1024] f32,
out [2,16,2048,1024] f32.  8 NeuronCores, data+head parallel: each core owns
4 (batch, head) pairs; V is shared across the 4 heads of a core's batch.

Math (reference semantics), per (b,h), chunks of 128:
  QR = rope(Q); KR = QR
  out_c = q_c @ state_{<c} + (q_c q_c^T  * strict_tril) v_c
  state += q_c^T v_c

Design:
  * RoPE is a fixed elementwise map of the input, so it is precomputed on
    the host (in fp32, then cast fp16) in the layouts the matmuls need.
  * Per (bh, superchunk) the transposed q (G lhsT/rhs + inter lhsT) and
    natural q (state-update lhsT) are packed in ONE 256KB DRAM piece with
    2KB-contiguous per-partition lines (1 descriptor per DMA engine), so
    each piece moves at wire speed with minimal latency; the first piece
    gates the first matmul at ~9.5us instead of 12.5us.
  * fp16 for all 16-bit work; fp8(e4m3) DoubleRow PV matmuls: per
    superchunk of SUP=2 chunks, the transposed score blocks G_j are
    evacuated into one [128, 2, 256] fp8 pair tile; the PV for chunk i is
    ONE DoubleRow matmul per D-half contracting 256 rows at 2x rate.
    The pair row of the later chunk j1 is zeroed in its leading 128 cols
    by a gpsimd memset (free engine), the rest by mask-multiplies on DVE.
  * PSUM-resident fp32 state; cast to fp16 SBUF right after each
    superchunk's accumulation group closes, split by m-plane across
    scalar (m0) and vector (m1) so the next superchunk's inter matmuls
    unblock as early as possible.
  * PSUM out evacuation split across scalar/vector by a 5:3 chunk
    pattern to balance the two PSUM-capable engines.
All DRAM layouts are partition-major; the output is written
partition-major and un-permuted on host.
"""

import numpy as np
import ml_dtypes

import concourse.mybir as mybir
import concourse.tile as tile
from concourse import bacc
from concourse.bass import ds
from concourse.bass_utils import run_bass_kernel_spmd

B, NH, T, N, D = 2, 16, 2048, 256, 1024
P = 128          # partition / chunk size
NCH = T // P     # 16 chunks
SUP = 2          # chunks per superchunk
NSUP = NCH // SUP
HPC = 4          # (b,h) pairs per core
NCORES = 8
THETA = 2.0 ** 16
TWO_PI = 2.0 * np.pi

f16 = mybir.dt.float16
f8 = mybir.dt.float8e4
f32 = mybir.dt.float32
f16_np = np.float16
f8_np = ml_dtypes.float8_e4m3  # TRN-style e4m3 (max normal 240)

mult = mybir.AluOpType.mult
DR = mybir.MatmulPerfMode.DoubleRow

# engine assignment knobs.
OUT_EVAC_ENG = ("s", "v")     # by chunk parity
STATE_CAST_ENG = ("s", "s")   # by m-plane


def _copy(nc, c, out, in_):
    if c == "s":
        nc.scalar.copy(out, in_)
    else:
        nc.vector.tensor_copy(out, in_)


def _emit_body(nc, tc, qX, v, v8, mskT, out):
    """Tile program for one core: 4 (b,h) pairs, full scan each."""
    with (
        tc.tile_pool(name="const", bufs=1) as constp,
        tc.tile_pool(name="qpool", bufs=2) as qpool,
        tc.tile_pool(name="work", bufs=4) as work,
        tc.tile_pool(name="outbuf", bufs=2) as outp,
        tc.tile_pool(name="statesb", bufs=2) as statep,
        tc.tile_pool(name="ps_state", bufs=1, space="PSUM") as ps_state,
        # one 4-bank ring shared by the per-chunk out tiles AND the
        # pipelined G tile (1 G per 4 out gens): decouples each tile's
        # first write from its predecessors' evacuation by ~2 chunks.
        tc.tile_pool(name="ps_out", bufs=4, space="PSUM") as ps_out,
    ):
        msk_sb = constp.tile([P, 2, 256], f16)
        v8_sb = constp.tile([P, NCH, D], f8)
        v_sb = constp.tile([P, NCH, D], f16)
        # Early DMA: each queue pays ~1-2.5us handoff latency per
        # dma_start before its packets flow, and pieces on one queue are
        # strictly chained -- so the prologue uses FEW, FAT pieces spread
        # over the three queues, ordered by first consumption.
        # scalar (first issue delayed ~1.3us by its ACT table load) gets
        # the mask (small) + mid-scan v; sync carries the early v chain.
        nc.scalar.dma_start(msk_sb[:], mskT[:, :, :])
        nc.scalar.dma_start(v8_sb[:, 4:8], v8[:, 4:8, :])
        nc.scalar.dma_start(v_sb[:, 4:8], v[:, 4:8, :])
        nc.sync.dma_start(v8_sb[:, 0:4], v8[:, 0:4, :])
        nc.sync.dma_start(v_sb[:, 0:2], v[:, 0:2, :])
        nc.sync.dma_start(v_sb[:, 2:4], v[:, 2:4, :])
        nc.sync.dma_start(v8_sb[:, 8:16], v8[:, 8:16, :])
        nc.sync.dma_start(v_sb[:, 8:12], v[:, 8:12, :])
        nc.sync.dma_start(v_sb[:, 12:16], v[:, 12:16, :])

        def bh_prologue(bh):
            """Allocate the per-bh q tile + emit per-superchunk DMAs."""
            q_sb = qpool.tile([P, NSUP, 1024], f16, tag="q", name=f"q{bh}")
            for s in range(NSUP):
                nc.gpsimd.dma_start(q_sb[:, s, :], qX[bh, s, :, :])
            return q_sb

        # within a sup piece: qt(m, t') at col m*256 + t'; qn(m, ci, k)
        # at col 512 + m*256 + ci*128 + k.
        def emit_G(q_tile, s):
            # Transposed score blocks for superchunk s's two chunks into
            # one PSUM tile: G_j0 at cols 0:256 (diag block + the j1
            # block), G_j1 at 256:384; then the masked fp8 evacuation
            # into a pair tile [p, j', i-col]: row 0 = G_j0 (diag-masked
            # then ones), row 1 = [zeros (gpsimd memset) | G_j1
            # diag-masked].  Called one superchunk AHEAD so the
            # evacuation is off the boundary's critical path.
            g_ps = ps_out.tile([P, 512], f32, tag="outp", name="g_ps")
            nc.tensor.matmul(
                g_ps[:, 0:256], q_tile[:, s, ds(0, 128)],
                q_tile[:, s, ds(0, 256)], start=True, stop=False,
            )
            nc.tensor.matmul(
                g_ps[:, 0:256], q_tile[:, s, ds(256, 128)],
                q_tile[:, s, ds(256, 256)], start=False, stop=True,
            )
            nc.tensor.matmul(
                g_ps[:, 256:384], q_tile[:, s, ds(128, 128)],
                q_tile[:, s, ds(128, 128)], start=True, stop=False,
            )
            nc.tensor.matmul(
                g_ps[:, 256:384], q_tile[:, s, ds(384, 128)],
                q_tile[:, s, ds(384, 128)], start=False, stop=True,
            )
            g2 = work.tile([P, 2, 256], f8, tag="g2", name="g2")
            nc.vector.tensor_tensor(g2[:, 0], g_ps[:, 0:256], msk_sb[:, 0], mult)
            nc.vector.tensor_tensor(g2[:, 1], g_ps[:, 128:384], msk_sb[:, 1], mult)
            return g2

        nxt = bh_prologue(0)
        g2_cur = emit_G(nxt, 0)
        for bh in range(HPC):
            q_sb = nxt
            nxt = None

            state_ps = ps_state.tile([P, 2, D], f32, tag="state")
            out_sbs = [
                outp.tile([P, NCH // 2, D], f16, tag=f"out{h}", name=f"out_sb{h}")
                for h in range(2)
            ]

            def emit_state_chunk(s, ci, i, is_start, is_close):
                # state += qr_c^T v_c (PSUM accumulate).  Each
                # superchunk's accumulation is a CLOSED group; on close,
                # each m-plane is cast fp32->fp16 on its own engine
                # (scalar m0 / vector m1) immediately, so the next
                # superchunk's inter matmuls unblock per-plane.
                sb = (
                    statep.tile([P, 2, D], f16, tag="state_sb", name="state_sb")
                    if is_close else None
                )
                for m in range(2):
                    for h in range(2):
                        dsl = ds(h * 512, 512)
                        nc.tensor.matmul(
                            state_ps[:, m, dsl],
                            q_sb[:, s, ds(512 + m * 256 + ci * 128, 128)],
                            v_sb[:, i, dsl],
                            start=is_start,
                            stop=is_close,
                            skip_group_check=True,
                        )
                    if is_close:
                        _copy(
                            nc, STATE_CAST_ENG[m], sb[:, m, :], state_ps[:, m, :]
                        )
                return sb

            state_sb_next = None
            for s in range(NSUP):
                state_sb = state_sb_next
                j0 = SUP * s
                g2 = g2_cur

                if bh < HPC - 1 and s == 4:
                    nxt = bh_prologue(bh + 1)

                for ci in range(SUP):
                    i = SUP * s + ci
                    if 0 < s < NSUP - 1:
                        # NB: matmul PSUM output is capped at 512 fp32
                        # (one bank) -> per-(m,h) matmuls.  State after
                        # the last superchunk is never read -> skipped.
                        sb = emit_state_chunk(s, ci, i, False, ci == SUP - 1)
                        if ci == SUP - 1:
                            state_sb_next = sb
                    out_ps = [
                        ps_out.tile([P, 512], f32, tag="outp", name=f"out_ps{h}")
                        for h in range(2)
                    ]
                    if s > 0:
                        # m-outer / h-inner: consecutive matmuls share lhsT
                        for m in range(2):
                            for h in range(2):
                                nc.tensor.matmul(
                                    out_ps[h][:],
                                    q_sb[:, s, ds(m * 256 + ci * 128, 128)],
                                    state_sb[:, m, ds(h * 512, 512)],
                                    start=(m == 0), stop=False,
                                    skip_group_check=True,
                                )
                    # PV: one fp8 DoubleRow matmul per D-half, contracting
                    # both chunks of the superchunk at 2x rate.
                    for h in range(2):
                        nc.tensor.matmul(
                            out_ps[h][:],
                            g2[:, :, ds(ci * P, P)],
                            v8_sb[:, ds(j0, SUP), ds(h * 512, 512)],
                            start=(s == 0), stop=True,
                            perf_mode=DR,
                            skip_group_check=True,
                        )

                    out_sb = out_sbs[i // (NCH // 2)]
                    if i == NCH - 1 and bh == HPC - 1:
                        engs = ("s", "v")  # final chunk: minimize drain latency
                    else:
                        e = OUT_EVAC_ENG[i % len(OUT_EVAC_ENG)]
                        engs = (e, e)
                    for h in range(2):
                        _copy(
                            nc, engs[h],
                            out_sb[:, i % (NCH // 2), ds(h * 512, 512)],
                            out_ps[h][:],
                        )
                    if ci == 0:
                        # pipeline: emit the NEXT superchunk's G + fp8 evac
                        # now (PE runs it after this sup's remaining work;
                        # the DVE evac lands before the next PV needs it)
                        if s + 1 < NSUP:
                            g2_cur = emit_G(q_sb, s + 1)
                        elif bh < HPC - 1:
                            g2_cur = emit_G(nxt, 0)
                    if s == NSUP - 1 and bh == HPC - 1:
                        # drain tail: per-chunk, D-halves on two different
                        # queues so the final transfers ride parallel rings
                        nc.sync.dma_start(
                            out[bh, :, ds(i, 1), ds(0, 512)],
                            out_sbs[i // (NCH // 2)][:, ds(i % (NCH // 2), 1), ds(0, 512)],
                        )
                        nc.scalar.dma_start(
                            out[bh, :, ds(i, 1), ds(512, 512)],
                            out_sbs[i // (NCH // 2)][:, ds(i % (NCH // 2), 1), ds(512, 512)],
                        )
                    elif ci == SUP - 1:
                        for cc in range(SUP):
                            nc.sync.dma_start(
                                out[bh, :, ds(j0 + cc, 1), :],
                                out_sbs[j0 // (NCH // 2)][
                                    :, ds((j0 + cc) % (NCH // 2), 1)
                                ],
                            )

                if s == 0:
                    # sup0's state is deferred past its PV so the PV
                    # starts ASAP after the prologue.
                    for ci2 in range(SUP):
                        sb = emit_state_chunk(
                            0, ci2, ci2, ci2 == 0, ci2 == SUP - 1
                        )
                        if ci2 == SUP - 1:
                            state_sb_next = sb


_BUILT = {}


def _build():
    if "nc" in _BUILT:
        return _BUILT["nc"]
    nc = bacc.Bacc(
        "TRN2", target_bir_lowering=False, debug=False,
        enable_asserts=True, num_devices=NCORES,
    )
    qX = nc.dram_tensor("q", [HPC, NSUP, P, 1024], f16, kind="ExternalInput")
    v = nc.dram_tensor("v", [P, NCH, D], f16, kind="ExternalInput")
    v8 = nc.dram_tensor("v8", [P, NCH, D], f8, kind="ExternalInput")
    mskT = nc.dram_tensor("mskT", [P, 2, SUP * P], f16, kind="ExternalInput")
    out = nc.dram_tensor("out", [HPC, P, NCH, D], f16, kind="ExternalOutput")
    with tile.TileContext(nc) as tc:
        _emit_body(nc, tc, qX, v, v8, mskT, out)
    nc.compile()
    _BUILT["nc"] = nc
    return nc


def _host_prep(Q_raw, V_raw):
    """Shard + precompute device inputs (fp16/fp8, partition-major),
    including the RoPE rotation (an input-only elementwise transform),
    computed in fp32 exactly as the reference does."""
    Q = np.asarray(Q_raw, dtype=np.float32)
    V = np.asarray(V_raw, dtype=np.float32)

    t = np.arange(N, dtype=np.float32)
    q = np.floor(t / 2.0) * 2.0
    freqs = (1.0 / (THETA ** (q / np.float32(N))) / np.float32(TWO_PI)).astype(
        np.float32
    )
    phases = np.arange(T, dtype=np.float32)[:, None] * freqs[None, :]
    ph = (phases % 1.0) * np.float32(TWO_PI)
    cosf = np.cos(ph).astype(np.float32)  # [T, N]
    sinf = np.sin(ph).astype(np.float32)
    QR = np.empty_like(Q)
    Qe, Qo = Q[..., 0::2], Q[..., 1::2]
    ce, se = cosf[:, 0::2], sinf[:, 0::2]
    QR[..., 0::2] = Qe * ce - Qo * se
    QR[..., 1::2] = Qo * ce + Qe * se

    # pair-tile masks [P, 2, 2P]: row 0 = [strict-triu | ones] (G_j0: diag
    # block then the full j1 block), row 1 = [zeros | strict-triu] (G_j1)
    mskT = np.zeros((P, 2, SUP * P), np.float32)
    mskT[:, 0, :P] = np.triu(np.ones((P, P), np.float32), k=1)
    mskT[:, 0, P:] = 1.0
    mskT[:, 1, P:] = np.triu(np.ones((P, P), np.float32), k=1)
    mskT = mskT.astype(f16_np)

    # deinterleave pairs: planes (evens, odds), cast fp16
    Qd = np.stack([QR[..., 0::2], QR[..., 1::2]], axis=2).astype(f16_np)
    # Qd: [B, NH, 2, T, 128]
    # per-(bh, sup) piece, 2KB contiguous per partition:
    #   cols [0:512]    qt: [m, t'] -> Qd[b,h,m, s*256 + t', k=p]
    #   cols [512:1024] qn: [m, ci, k] -> Qd[b,h,m, s*256+ci*128+p, k]
    Qt = (
        Qd.transpose(0, 1, 4, 2, 3)              # [B,NH,k,2,T]
        .reshape(B, NH, P, 2, NSUP, SUP * P)
        .transpose(0, 1, 4, 2, 3, 5)             # [B,NH,NSUP,k,2,256]
        .reshape(B, NH, NSUP, P, 512)
    )
    Qn = (
        Qd.reshape(B, NH, 2, NSUP, SUP, P, P)    # [B,NH,m,s,ci,t',k]
        .transpose(0, 1, 3, 5, 2, 4, 6)          # [B,NH,s,t',m,ci,k]
        .reshape(B, NH, NSUP, P, 512)
    )
    q2 = np.ascontiguousarray(np.concatenate([Qt, Qn], axis=-1))
    # q2: [B, NH, NSUP, P, 1024]

    V16 = V.astype(f16_np)
    # v layout [P, NCH, D]: (p, c, d) = V[c*128+p, d]
    Vp = np.ascontiguousarray(V16.reshape(B, NCH, P, D).transpose(0, 2, 1, 3))
    V8p = Vp.astype(f8_np)

    in_maps = []
    for core in range(NCORES):
        b = core // (NCORES // B)
        hs = (core % (NCORES // B)) * HPC
        in_maps.append(
            {
                "q": np.ascontiguousarray(q2[b, hs : hs + HPC]),
                "v": Vp[b],
                "v8": V8p[b],
                "mskT": mskT,
            }
        )
    return in_maps


def _run(inputs, trace=False, **kw):
    nc = _build()
    in_maps = _host_prep(inputs["Q_raw"], inputs["V_raw"])
    res = run_bass_kernel_spmd(nc, in_maps, list(range(NCORES)), trace=trace, **kw)
    out = np.empty((B, NH, T, D), dtype=np.float32)
    for core in range(NCORES):
        b = core // (NCORES // B)
        hs = (core % (NCORES // B)) * HPC
        # device out: [HPC, P, NCH, D] partition-major -> [HPC, T, D]
        o = res.results[core]["out"].astype(np.float32)
        out[b, hs : hs + HPC] = o.transpose(0, 2, 1, 3).reshape(HPC, T, D)
    return out, res


def kernel(**inputs):
    out, _ = _run(inputs)
    return out


# revision 16
# speedup vs baseline: 1.0743x; 1.0274x over previous
"""Trainium2 Bass kernel for BDH recurrent (chunked linear) attention.

Problem shapes (hardcoded): Q_raw [2,16,2048,256] f32, V_raw [2,2048,1024] f32,
out [2,16,2048,1024] f32.  8 NeuronCores, data+head parallel: each core owns
4 (batch, head) pairs; V is shared across the 4 heads of a core's batch.

Math (reference semantics), per (b,h), chunks of 128:
  QR = rope(Q); KR = QR
  out_c = q_c @ state_{<c} + (q_c q_c^T  * strict_tril) v_c
  state += q_c^T v_c

Design:
  * RoPE is a fixed elementwise map of the input, so it is precomputed on
    the host (in fp32, then cast fp16) in the layouts the matmuls need.
  * Per (bh, superchunk) the transposed q (G lhsT/rhs + inter lhsT) and
    natural q (state-update lhsT) are packed in ONE 256KB DRAM piece with
    2KB-contiguous per-partition lines (1 descriptor per DMA engine), so
    each piece moves at wire speed with minimal latency; the first piece
    gates the first matmul at ~9.5us instead of 12.5us.
  * fp16 for all 16-bit work; fp8(e4m3) DoubleRow PV matmuls: per
    superchunk of SUP=2 chunks, the transposed score blocks G_j are
    evacuated into one [128, 2, 256] fp8 pair tile; the PV for chunk i is
    ONE DoubleRow matmul per D-half contracting 256 rows at 2x rate.
    The pair row of the later chunk j1 is zeroed in its leading 128 cols
    by a gpsimd memset (free engine), the rest by mask-multiplies on DVE.
  * PSUM-resident fp32 state; cast to fp16 SBUF right after each
    superchunk's accumulation group closes, split by m-plane across
    scalar (m0) and vector (m1) so the next superchunk's inter matmuls
    unblock as early as possible.
  * PSUM out evacuation split across scalar/vector by a 5:3 chunk
    pattern to balance the two PSUM-capable engines.
All DRAM layouts are partition-major; the output is written
partition-major and un-permuted on host.
"""

import numpy as np
import ml_dtypes

import concourse.mybir as mybir
import concourse.tile as tile
from concourse import bacc
from concourse.bass import ds
from concourse.bass_utils import run_bass_kernel_spmd

B, NH, T, N, D = 2, 16, 2048, 256, 1024
P = 128          # partition / chunk size
NCH = T // P     # 16 chunks
SUP = 2          # chunks per superchunk
NSUP = NCH // SUP
HPC = 4          # (b,h) pairs per core
NCORES = 8
THETA = 2.0 ** 16
TWO_PI = 2.0 * np.pi

f16 = mybir.dt.float16
f8 = mybir.dt.float8e4
f32 = mybir.dt.float32
f16_np = np.float16
f8_np = ml_dtypes.float8_e4m3  # TRN-style e4m3 (max normal 240)

mult = mybir.AluOpType.mult
DR = mybir.MatmulPerfMode.DoubleRow

# engine assignment knobs.
OUT_EVAC_ENG = ("s", "v")     # by chunk parity
STATE_CAST_ENG = ("s", "s")   # by m-plane


def _copy(nc, c, out, in_):
    if c == "s":
        nc.scalar.copy(out, in_)
    else:
        nc.vector.tensor_copy(out, in_)


def _emit_body(nc, tc, qX, v, v8, mskT, out):
    """Tile program for one core: 4 (b,h) pairs, full scan each."""
    with (
        tc.tile_pool(name="const", bufs=1) as constp,
        tc.tile_pool(name="qpool", bufs=2) as qpool,
        tc.tile_pool(name="work", bufs=4) as work,
        tc.tile_pool(name="outbuf", bufs=2) as outp,
        tc.tile_pool(name="statesb", bufs=2) as statep,
        tc.tile_pool(name="ps_state", bufs=1, space="PSUM") as ps_state,
        tc.tile_pool(name="ps_out", bufs=3, space="PSUM") as ps_out,
        tc.tile_pool(name="ps_g", bufs=1, space="PSUM") as ps_g,
    ):
        msk_sb = constp.tile([P, 2, 256], f16)
        v8_sb = constp.tile([P, NCH, D], f8)
        v_sb = constp.tile([P, NCH, D], f16)
        # Early DMA: the 16 hardware DMA engines are a SHARED pool that
        # serves all queues roughly in doorbell order, so the effective
        # prologue latency is the running SUM of issued piece sizes.
        # Keep the critical set (mask, v8/v of the first chunks, q sup0
        # on gpsimd) small and first; everything else follows.  scalar's
        # first issue is delayed ~1.3us by its ACT table load, so it
        # only gets mid-scan pieces.
        nc.sync.dma_start(msk_sb[:], mskT[:, :, :])
        nc.sync.dma_start(v8_sb[:, 0:2], v8[:, 0:2, :])
        nc.sync.dma_start(v_sb[:, 0:2], v[:, 0:2, :])
        nc.sync.dma_start(v8_sb[:, 2:4], v8[:, 2:4, :])
        nc.sync.dma_start(v_sb[:, 2:4], v[:, 2:4, :])
        nc.scalar.dma_start(v8_sb[:, 4:8], v8[:, 4:8, :])
        nc.scalar.dma_start(v_sb[:, 4:8], v[:, 4:8, :])
        nc.scalar.dma_start(v8_sb[:, 8:16], v8[:, 8:16, :])
        nc.scalar.dma_start(v_sb[:, 8:12], v[:, 8:12, :])
        nc.scalar.dma_start(v_sb[:, 12:16], v[:, 12:16, :])

        def bh_prologue(bh):
            """Allocate the per-bh q tile + emit per-superchunk DMAs."""
            q_sb = qpool.tile([P, NSUP, 1024], f16, tag="q", name=f"q{bh}")
            for s in range(NSUP):
                nc.gpsimd.dma_start(q_sb[:, s, :], qX[bh, s, :, :])
            return q_sb

        # within a sup piece: qt(m, t') at col m*256 + t'; qn(m, ci, k)
        # at col 512 + m*256 + ci*128 + k.
        def emit_G(q_tile, s):
            # Transposed score blocks for superchunk s's two chunks into
            # one PSUM tile: G_j0 at cols 0:256 (diag block + the j1
            # block), G_j1 at 256:384; then the masked fp8 evacuation
            # into a pair tile [p, j', i-col]: row 0 = G_j0 (diag-masked
            # then ones), row 1 = [zeros (gpsimd memset) | G_j1
            # diag-masked].  Called one superchunk AHEAD so the
            # evacuation is off the boundary's critical path.
            g_ps = ps_g.tile([P, 384], f32, tag="g", name="g_ps")
            nc.tensor.matmul(
                g_ps[:, 0:256], q_tile[:, s, ds(0, 128)],
                q_tile[:, s, ds(0, 256)], start=True, stop=False,
            )
            nc.tensor.matmul(
                g_ps[:, 0:256], q_tile[:, s, ds(256, 128)],
                q_tile[:, s, ds(256, 256)], start=False, stop=True,
            )
            nc.tensor.matmul(
                g_ps[:, 256:384], q_tile[:, s, ds(128, 128)],
                q_tile[:, s, ds(128, 128)], start=True, stop=False,
            )
            nc.tensor.matmul(
                g_ps[:, 256:384], q_tile[:, s, ds(384, 128)],
                q_tile[:, s, ds(384, 128)], start=False, stop=True,
            )
            g2 = work.tile([P, 2, 256], f8, tag="g2", name="g2")
            nc.vector.tensor_tensor(g2[:, 0], g_ps[:, 0:256], msk_sb[:, 0], mult)
            nc.vector.tensor_tensor(g2[:, 1], g_ps[:, 128:384], msk_sb[:, 1], mult)
            return g2

        nxt = bh_prologue(0)
        g2_cur = emit_G(nxt, 0)
        for bh in range(HPC):
            q_sb = nxt
            nxt = None

            state_ps = ps_state.tile([P, 2, D], f32, tag="state")
            out_sbs = [
                outp.tile([P, NCH // 2, D], f16, tag=f"out{h}", name=f"out_sb{h}")
                for h in range(2)
            ]

            def emit_state_chunk(s, ci, i, is_start, is_close):
                # state += qr_c^T v_c (PSUM accumulate).  Each
                # superchunk's accumulation is a CLOSED group; on close,
                # each m-plane is cast fp32->fp16 on its own engine
                # (scalar m0 / vector m1) immediately, so the next
                # superchunk's inter matmuls unblock per-plane.
                sb = (
                    statep.tile([P, 2, D], f16, tag="state_sb", name="state_sb")
                    if is_close else None
                )
                for m in range(2):
                    for h in range(2):
                        dsl = ds(h * 512, 512)
                        nc.tensor.matmul(
                            state_ps[:, m, dsl],
                            q_sb[:, s, ds(512 + m * 256 + ci * 128, 128)],
                            v_sb[:, i, dsl],
                            start=is_start,
                            stop=is_close,
                            skip_group_check=True,
                        )
                    if is_close:
                        _copy(
                            nc, STATE_CAST_ENG[m], sb[:, m, :], state_ps[:, m, :]
                        )
                return sb

            state_sb_next = None
            for s in range(NSUP):
                state_sb = state_sb_next
                j0 = SUP * s
                g2 = g2_cur

                if bh < HPC - 1 and s == 4:
                    nxt = bh_prologue(bh + 1)

                # Emit the whole superchunk's state matmuls + the closing
                # casts BEFORE the out-blocks: the casts then precede the
                # out evacuations in scalar's strict FIFO, landing ~2us
                # before the next superchunk's inter matmuls need them
                # (emitting them after the evacs cost ~600ns of PE stall
                # per superchunk).  State after the last superchunk is
                # never read -> skipped.  bh0's sup0 is deferred below so
                # the first PV isn't queued behind it during the DMA
                # prologue.
                if s < NSUP - 1 and not (s == 0 and bh == 0):
                    for ci in range(SUP):
                        sb = emit_state_chunk(
                            s, ci, SUP * s + ci,
                            s == 0 and ci == 0, ci == SUP - 1,
                        )
                    state_sb_next = sb

                for ci in range(SUP):
                    i = SUP * s + ci
                    out_ps = [
                        ps_out.tile([P, 512], f32, tag="outp", name=f"out_ps{h}")
                        for h in range(2)
                    ]
                    if s > 0:
                        # m-outer / h-inner: consecutive matmuls share lhsT
                        for m in range(2):
                            for h in range(2):
                                nc.tensor.matmul(
                                    out_ps[h][:],
                                    q_sb[:, s, ds(m * 256 + ci * 128, 128)],
                                    state_sb[:, m, ds(h * 512, 512)],
                                    start=(m == 0), stop=False,
                                    skip_group_check=True,
                                )
                    # PV: one fp8 DoubleRow matmul per D-half, contracting
                    # both chunks of the superchunk at 2x rate.
                    for h in range(2):
                        nc.tensor.matmul(
                            out_ps[h][:],
                            g2[:, :, ds(ci * P, P)],
                            v8_sb[:, ds(j0, SUP), ds(h * 512, 512)],
                            start=(s == 0), stop=True,
                            perf_mode=DR,
                            skip_group_check=True,
                        )

                    out_sb = out_sbs[i // (NCH // 2)]
                    if i == NCH - 1 and bh == HPC - 1:
                        engs = ("s", "v")  # final chunk: minimize drain latency
                    else:
                        e = OUT_EVAC_ENG[i % len(OUT_EVAC_ENG)]
                        engs = (e, e)
                    for h in range(2):
                        _copy(
                            nc, engs[h],
                            out_sb[:, i % (NCH // 2), ds(h * 512, 512)],
                            out_ps[h][:],
                        )
                    if ci == 0:
                        # pipeline: emit the NEXT superchunk's G + fp8 evac
                        # now (PE runs it after this sup's remaining work;
                        # the DVE evac lands before the next PV needs it)
                        if s + 1 < NSUP:
                            g2_cur = emit_G(q_sb, s + 1)
                        elif bh < HPC - 1:
                            g2_cur = emit_G(nxt, 0)
                    if s == NSUP - 1 and bh == HPC - 1:
                        # drain tail: per-chunk, D-halves on two different
                        # queues so the final transfers ride parallel rings
                        nc.sync.dma_start(
                            out[bh, :, ds(i, 1), ds(0, 512)],
                            out_sbs[i // (NCH // 2)][:, ds(i % (NCH // 2), 1), ds(0, 512)],
                        )
                        nc.scalar.dma_start(
                            out[bh, :, ds(i, 1), ds(512, 512)],
                            out_sbs[i // (NCH // 2)][:, ds(i % (NCH // 2), 1), ds(512, 512)],
                        )
                    elif ci == SUP - 1:
                        for cc in range(SUP):
                            nc.sync.dma_start(
                                out[bh, :, ds(j0 + cc, 1), :],
                                out_sbs[j0 // (NCH // 2)][
                                    :, ds((j0 + cc) % (NCH // 2), 1)
                                ],
                            )

                if s == 0 and bh == 0:
                    # bh0 sup0's state is deferred past its PV so the PV
                    # starts ASAP after the prologue.
                    for ci2 in range(SUP):
                        sb = emit_state_chunk(
                            0, ci2, ci2, ci2 == 0, ci2 == SUP - 1
                        )
                    state_sb_next = sb


_BUILT = {}


def _build():
    if "nc" in _BUILT:
        return _BUILT["nc"]
    nc = bacc.Bacc(
        "TRN2", target_bir_lowering=False, debug=False,
        enable_asserts=True, num_devices=NCORES,
    )
    qX = nc.dram_tensor("q", [HPC, NSUP, P, 1024], f16, kind="ExternalInput")
    v = nc.dram_tensor("v", [P, NCH, D], f16, kind="ExternalInput")
    v8 = nc.dram_tensor("v8", [P, NCH, D], f8, kind="ExternalInput")
    mskT = nc.dram_tensor("mskT", [P, 2, SUP * P], f16, kind="ExternalInput")
    out = nc.dram_tensor("out", [HPC, P, NCH, D], f16, kind="ExternalOutput")
    with tile.TileContext(nc) as tc:
        _emit_body(nc, tc, qX, v, v8, mskT, out)
    nc.compile()
    _BUILT["nc"] = nc
    return nc


def _host_prep(Q_raw, V_raw):
    """Shard + precompute device inputs (fp16/fp8, partition-major),
    including the RoPE rotation (an input-only elementwise transform),
    computed in fp32 exactly as the reference does."""
    Q = np.asarray(Q_raw, dtype=np.float32)
    V = np.asarray(V_raw, dtype=np.float32)

    t = np.arange(N, dtype=np.float32)
    q = np.floor(t / 2.0) * 2.0
    freqs = (1.0 / (THETA ** (q / np.float32(N))) / np.float32(TWO_PI)).astype(
        np.float32
    )
    phases = np.arange(T, dtype=np.float32)[:, None] * freqs[None, :]
    ph = (phases % 1.0) * np.float32(TWO_PI)
    cosf = np.cos(ph).astype(np.float32)  # [T, N]
    sinf = np.sin(ph).astype(np.float32)
    QR = np.empty_like(Q)
    Qe, Qo = Q[..., 0::2], Q[..., 1::2]
    ce, se = cosf[:, 0::2], sinf[:, 0::2]
    QR[..., 0::2] = Qe * ce - Qo * se
    QR[..., 1::2] = Qo * ce + Qe * se

    # pair-tile masks [P, 2, 2P]: row 0 = [strict-triu | ones] (G_j0: diag
    # block then the full j1 block), row 1 = [zeros | strict-triu] (G_j1)
    mskT = np.zeros((P, 2, SUP * P), np.float32)
    mskT[:, 0, :P] = np.triu(np.ones((P, P), np.float32), k=1)
    mskT[:, 0, P:] = 1.0
    mskT[:, 1, P:] = np.triu(np.ones((P, P), np.float32), k=1)
    mskT = mskT.astype(f16_np)

    # deinterleave pairs: planes (evens, odds), cast fp16
    Qd = np.stack([QR[..., 0::2], QR[..., 1::2]], axis=2).astype(f16_np)
    # Qd: [B, NH, 2, T, 128]
    # per-(bh, sup) piece, 2KB contiguous per partition:
    #   cols [0:512]    qt: [m, t'] -> Qd[b,h,m, s*256 + t', k=p]
    #   cols [512:1024] qn: [m, ci, k] -> Qd[b,h,m, s*256+ci*128+p, k]
    Qt = (
        Qd.transpose(0, 1, 4, 2, 3)              # [B,NH,k,2,T]
        .reshape(B, NH, P, 2, NSUP, SUP * P)
        .transpose(0, 1, 4, 2, 3, 5)             # [B,NH,NSUP,k,2,256]
        .reshape(B, NH, NSUP, P, 512)
    )
    Qn = (
        Qd.reshape(B, NH, 2, NSUP, SUP, P, P)    # [B,NH,m,s,ci,t',k]
        .transpose(0, 1, 3, 5, 2, 4, 6)          # [B,NH,s,t',m,ci,k]
        .reshape(B, NH, NSUP, P, 512)
    )
    q2 = np.ascontiguousarray(np.concatenate([Qt, Qn], axis=-1))
    # q2: [B, NH, NSUP, P, 1024]

    V16 = V.astype(f16_np)
    # v layout [P, NCH, D]: (p, c, d) = V[c*128+p, d]
    Vp = np.ascontiguousarray(V16.reshape(B, NCH, P, D).transpose(0, 2, 1, 3))
    V8p = Vp.astype(f8_np)

    in_maps = []
    for core in range(NCORES):
        b = core // (NCORES // B)
        hs = (core % (NCORES // B)) * HPC
        in_maps.append(
            {
                "q": np.ascontiguousarray(q2[b, hs : hs + HPC]),
                "v": Vp[b],
                "v8": V8p[b],
                "mskT": mskT,
            }
        )
    return in_maps


def _run(inputs, trace=False, **kw):
    nc = _build()
    in_maps = _host_prep(inputs["Q_raw"], inputs["V_raw"])
    res = run_bass_kernel_spmd(nc, in_maps, list(range(NCORES)), trace=trace, **kw)
    out = np.empty((B, NH, T, D), dtype=np.float32)
    for core in range(NCORES):
        b = core // (NCORES // B)
        hs = (core % (NCORES // B)) * HPC
        # device out: [HPC, P, NCH, D] partition-major -> [HPC, T, D]
        o = res.results[core]["out"].astype(np.float32)
        out[b, hs : hs + HPC] = o.transpose(0, 2, 1, 3).reshape(HPC, T, D)
    return out, res


def kernel(**inputs):
    out, _ = _run(inputs)
    return out


# revision 20
# speedup vs baseline: 1.1099x; 1.0331x over previous
"""Trainium2 Bass kernel for BDH recurrent (chunked linear) attention.

Problem shapes (hardcoded): Q_raw [2,16,2048,256] f32, V_raw [2,2048,1024] f32,
out [2,16,2048,1024] f32.  8 NeuronCores, data+head parallel: each core owns
4 (batch, head) pairs; V is shared across the 4 heads of a core's batch.

Math (reference semantics), per (b,h), chunks of 128:
  QR = rope(Q); KR = QR
  out_c = q_c @ state_{<c} + (q_c q_c^T  * strict_tril) v_c
  state += q_c^T v_c

Design:
  * RoPE is a fixed elementwise map of the input, so it is precomputed on
    the host (in fp32, then cast fp16) in the layouts the matmuls need.
  * Per (bh, superchunk) the transposed q (G lhsT/rhs + inter lhsT) and
    natural q (state-update lhsT) are packed in ONE 256KB DRAM piece with
    2KB-contiguous per-partition lines (1 descriptor per DMA engine), so
    each piece moves at wire speed with minimal latency; the first piece
    gates the first matmul at ~9.5us instead of 12.5us.
  * fp16 for all 16-bit work; fp8(e4m3) DoubleRow PV matmuls: per
    superchunk of SUP=2 chunks, the transposed score blocks G_j are
    evacuated into one [128, 2, 256] fp8 pair tile; the PV for chunk i is
    ONE DoubleRow matmul per D-half contracting 256 rows at 2x rate.
    The pair row of the later chunk j1 is zeroed in its leading 128 cols
    by a gpsimd memset (free engine), the rest by mask-multiplies on DVE.
  * PSUM-resident fp32 state; cast to fp16 SBUF right after each
    superchunk's accumulation group closes, split by m-plane across
    scalar (m0) and vector (m1) so the next superchunk's inter matmuls
    unblock as early as possible.
  * PSUM out evacuation split across scalar/vector by a 5:3 chunk
    pattern to balance the two PSUM-capable engines.
All DRAM layouts are partition-major; the output is written
partition-major and un-permuted on host.
"""

import numpy as np
import ml_dtypes

import concourse.mybir as mybir
import concourse.tile as tile
from concourse import bacc
from concourse.bass import ds
from concourse.bass_utils import run_bass_kernel_spmd

B, NH, T, N, D = 2, 16, 2048, 256, 1024
P = 128          # partition / chunk size
NCH = T // P     # 16 chunks
SUP = 2          # chunks per superchunk
NSUP = NCH // SUP
HPC = 4          # (b,h) pairs per core
NCORES = 8
THETA = 2.0 ** 16
TWO_PI = 2.0 * np.pi

f16 = mybir.dt.float16
f8 = mybir.dt.float8e4
f32 = mybir.dt.float32
f16_np = np.float16
f8_np = ml_dtypes.float8_e4m3  # TRN-style e4m3 (max normal 240)

mult = mybir.AluOpType.mult
DR = mybir.MatmulPerfMode.DoubleRow

# engine assignment knobs.
OUT_EVAC_ENG = ("s", "v")     # by chunk parity
STATE_CAST_ENG = ("s", "s")   # by m-plane


def _copy(nc, c, out, in_):
    if c == "s":
        nc.scalar.copy(out, in_)
    else:
        nc.vector.tensor_copy(out, in_)


def _emit_body(nc, tc, qX, v, v8, mskT, out):
    """Tile program for one core: 4 (b,h) pairs, full scan each."""
    with (
        tc.tile_pool(name="const", bufs=1) as constp,
        tc.tile_pool(name="qpool", bufs=2) as qpool,
        tc.tile_pool(name="work", bufs=4) as work,
        tc.tile_pool(name="outbuf", bufs=2) as outp,
        tc.tile_pool(name="statesb", bufs=2) as statep,
        tc.tile_pool(name="ps_state", bufs=1, space="PSUM") as ps_state,
        tc.tile_pool(name="ps_out", bufs=3, space="PSUM") as ps_out,
        tc.tile_pool(name="ps_g", bufs=1, space="PSUM") as ps_g,
    ):
        msk_sb = constp.tile([P, 2, 256], f16)
        v8_sb = constp.tile([P, NCH, D], f8)
        v_sb = constp.tile([P, NCH, D], f16)
        # DMA model: the 16 hardware DMA engines are a SHARED pool that
        # serves all queues roughly in doorbell (issue) order, and bh0's
        # scan demands ~300GB/s (v 6MB + q 2MB + out 4MB over ~40us) --
        # right at the pool's capacity.  So pieces are issued JUST IN
        # TIME, in consumption order: only the critical set up front,
        # everything else from inside the scan loop.  scalar's first
        # issue is delayed ~1.3us by its ACT table load, so it only gets
        # later pieces.
        nc.sync.dma_start(msk_sb[:], mskT[:, :, :])
        nc.sync.dma_start(v8_sb[:, 0:2], v8[:, 0:2, :])
        nc.sync.dma_start(v_sb[:, 0:2], v[:, 0:2, :])

        def emit_v_pair(k):
            # v8/v chunks [2k, 2k+2): v8 on scalar, v on sync
            nc.scalar.dma_start(v8_sb[:, 2 * k : 2 * k + 2], v8[:, 2 * k : 2 * k + 2, :])
            nc.sync.dma_start(v_sb[:, 2 * k : 2 * k + 2], v[:, 2 * k : 2 * k + 2, :])

        def q_piece(q_sb, bh, s):
            nc.gpsimd.dma_start(q_sb[:, s, :], qX[bh, s, :, :])

        def bh_prologue(bh, nsup0=2):
            """Allocate the per-bh q tile + DMA its first nsup0 pieces."""
            q_sb = qpool.tile([P, NSUP, 1024], f16, tag="q", name=f"q{bh}")
            for s in range(nsup0):
                q_piece(q_sb, bh, s)
            return q_sb

        # within a sup piece: qt(m, t') at col m*256 + t'; qn(m, ci, k)
        # at col 512 + m*256 + ci*128 + k.
        def emit_G(q_tile, s):
            # Transposed score blocks for superchunk s's two chunks into
            # one PSUM tile: G_j0 at cols 0:256 (diag block + the j1
            # block), G_j1 at 256:384; then the masked fp8 evacuation
            # into a pair tile [p, j', i-col]: row 0 = G_j0 (diag-masked
            # then ones), row 1 = [zeros (gpsimd memset) | G_j1
            # diag-masked].  Called one superchunk AHEAD so the
            # evacuation is off the boundary's critical path.
            g_ps = ps_g.tile([P, 384], f32, tag="g", name="g_ps")
            nc.tensor.matmul(
                g_ps[:, 0:256], q_tile[:, s, ds(0, 128)],
                q_tile[:, s, ds(0, 256)], start=True, stop=False,
            )
            nc.tensor.matmul(
                g_ps[:, 0:256], q_tile[:, s, ds(256, 128)],
                q_tile[:, s, ds(256, 256)], start=False, stop=True,
            )
            nc.tensor.matmul(
                g_ps[:, 256:384], q_tile[:, s, ds(128, 128)],
                q_tile[:, s, ds(128, 128)], start=True, stop=False,
            )
            nc.tensor.matmul(
                g_ps[:, 256:384], q_tile[:, s, ds(384, 128)],
                q_tile[:, s, ds(384, 128)], start=False, stop=True,
            )
            g2 = work.tile([P, 2, 256], f8, tag="g2", name="g2")
            nc.vector.tensor_tensor(g2[:, 0], g_ps[:, 0:256], msk_sb[:, 0], mult)
            nc.vector.tensor_tensor(g2[:, 1], g_ps[:, 128:384], msk_sb[:, 1], mult)
            return g2

        nxt = bh_prologue(0)
        g2_cur = emit_G(nxt, 0)
        for bh in range(HPC):
            q_sb = nxt
            nxt = None

            state_ps = ps_state.tile([P, 2, D], f32, tag="state")
            out_sbs = [
                outp.tile([P, NCH // 2, D], f16, tag=f"out{h}", name=f"out_sb{h}")
                for h in range(2)
            ]

            def emit_state_chunk(s, ci, i, is_start, is_close):
                # state += qr_c^T v_c (PSUM accumulate).  Each
                # superchunk's accumulation is a CLOSED group; on close,
                # each m-plane is cast fp32->fp16 on its own engine
                # (scalar m0 / vector m1) immediately, so the next
                # superchunk's inter matmuls unblock per-plane.
                sb = (
                    statep.tile([P, 2, D], f16, tag="state_sb", name="state_sb")
                    if is_close else None
                )
                for m in range(2):
                    for h in range(2):
                        dsl = ds(h * 512, 512)
                        nc.tensor.matmul(
                            state_ps[:, m, dsl],
                            q_sb[:, s, ds(512 + m * 256 + ci * 128, 128)],
                            v_sb[:, i, dsl],
                            start=is_start,
                            stop=is_close,
                            skip_group_check=True,
                        )
                    if is_close:
                        _copy(
                            nc, STATE_CAST_ENG[m], sb[:, m, :], state_ps[:, m, :]
                        )
                return sb

            state_sb_next = None
            for s in range(NSUP):
                state_sb = state_sb_next
                j0 = SUP * s
                g2 = g2_cur

                # JIT DMA issue, ~2 superchunks ahead of consumption:
                # own q piece s+2, then the next bh's first pieces; v
                # pairs (resident after bh0) one superchunk ahead.
                if s + 2 < NSUP:
                    q_piece(q_sb, bh, s + 2)
                elif bh < HPC - 1:
                    if s == NSUP - 2:
                        nxt = bh_prologue(bh + 1, nsup0=0)
                    q_piece(nxt, bh + 1, s + 2 - NSUP)
                if bh == 0 and s + 1 < NSUP:
                    emit_v_pair(s + 1)

                # Emit the whole superchunk's state matmuls + the closing
                # casts BEFORE the out-blocks: the casts then precede the
                # out evacuations in scalar's strict FIFO, landing ~2us
                # before the next superchunk's inter matmuls need them
                # (emitting them after the evacs cost ~600ns of PE stall
                # per superchunk).  State after the last superchunk is
                # never read -> skipped.  bh0's sup0 is deferred below so
                # the first PV isn't queued behind it during the DMA
                # prologue.
                if s < NSUP - 1 and not (s == 0 and bh == 0):
                    for ci in range(SUP):
                        sb = emit_state_chunk(
                            s, ci, SUP * s + ci,
                            s == 0 and ci == 0, ci == SUP - 1,
                        )
                    state_sb_next = sb

                for ci in range(SUP):
                    i = SUP * s + ci
                    out_ps = [
                        ps_out.tile([P, 512], f32, tag="outp", name=f"out_ps{h}")
                        for h in range(2)
                    ]
                    if s > 0:
                        # m-outer / h-inner: consecutive matmuls share lhsT
                        for m in range(2):
                            for h in range(2):
                                nc.tensor.matmul(
                                    out_ps[h][:],
                                    q_sb[:, s, ds(m * 256 + ci * 128, 128)],
                                    state_sb[:, m, ds(h * 512, 512)],
                                    start=(m == 0), stop=False,
                                    skip_group_check=True,
                                )
                    # PV: one fp8 DoubleRow matmul per D-half, contracting
                    # both chunks of the superchunk at 2x rate.
                    for h in range(2):
                        nc.tensor.matmul(
                            out_ps[h][:],
                            g2[:, :, ds(ci * P, P)],
                            v8_sb[:, ds(j0, SUP), ds(h * 512, 512)],
                            start=(s == 0), stop=True,
                            perf_mode=DR,
                            skip_group_check=True,
                        )

                    out_sb = out_sbs[i // (NCH // 2)]
                    if i == NCH - 1 and bh == HPC - 1:
                        engs = ("s", "v")  # final chunk: minimize drain latency
                    else:
                        e = OUT_EVAC_ENG[i % len(OUT_EVAC_ENG)]
                        engs = (e, e)
                    for h in range(2):
                        _copy(
                            nc, engs[h],
                            out_sb[:, i % (NCH // 2), ds(h * 512, 512)],
                            out_ps[h][:],
                        )
                    if ci == 0:
                        # pipeline: emit the NEXT superchunk's G + fp8 evac
                        # now (PE runs it after this sup's remaining work;
                        # the DVE evac lands before the next PV needs it)
                        if s + 1 < NSUP:
                            g2_cur = emit_G(q_sb, s + 1)
                        elif bh < HPC - 1:
                            g2_cur = emit_G(nxt, 0)
                    if s == NSUP - 1 and bh == HPC - 1:
                        # drain tail: per-chunk, D-halves on two different
                        # queues so the final transfers ride parallel rings
                        nc.sync.dma_start(
                            out[bh, :, ds(i, 1), ds(0, 512)],
                            out_sbs[i // (NCH // 2)][:, ds(i % (NCH // 2), 1), ds(0, 512)],
                        )
                        nc.scalar.dma_start(
                            out[bh, :, ds(i, 1), ds(512, 512)],
                            out_sbs[i // (NCH // 2)][:, ds(i % (NCH // 2), 1), ds(512, 512)],
                        )
                    elif ci == SUP - 1:
                        for cc in range(SUP):
                            nc.sync.dma_start(
                                out[bh, :, ds(j0 + cc, 1), :],
                                out_sbs[j0 // (NCH // 2)][
                                    :, ds((j0 + cc) % (NCH // 2), 1)
                                ],
                            )

                if s == 0 and bh == 0:
                    # bh0 sup0's state is deferred past its PV so the PV
                    # starts ASAP after the prologue.
                    for ci2 in range(SUP):
                        sb = emit_state_chunk(
                            0, ci2, ci2, ci2 == 0, ci2 == SUP - 1
                        )
                    state_sb_next = sb


_BUILT = {}


def _build():
    if "nc" in _BUILT:
        return _BUILT["nc"]
    nc = bacc.Bacc(
        "TRN2", target_bir_lowering=False, debug=False,
        enable_asserts=True, num_devices=NCORES,
    )
    qX = nc.dram_tensor("q", [HPC, NSUP, P, 1024], f16, kind="ExternalInput")
    v = nc.dram_tensor("v", [P, NCH, D], f16, kind="ExternalInput")
    v8 = nc.dram_tensor("v8", [P, NCH, D], f8, kind="ExternalInput")
    mskT = nc.dram_tensor("mskT", [P, 2, SUP * P], f16, kind="ExternalInput")
    out = nc.dram_tensor("out", [HPC, P, NCH, D], f16, kind="ExternalOutput")
    with tile.TileContext(nc) as tc:
        _emit_body(nc, tc, qX, v, v8, mskT, out)
    nc.compile()
    _BUILT["nc"] = nc
    return nc


def _host_prep(Q_raw, V_raw):
    """Shard + precompute device inputs (fp16/fp8, partition-major),
    including the RoPE rotation (an input-only elementwise transform),
    computed in fp32 exactly as the reference does."""
    Q = np.asarray(Q_raw, dtype=np.float32)
    V = np.asarray(V_raw, dtype=np.float32)

    t = np.arange(N, dtype=np.float32)
    q = np.floor(t / 2.0) * 2.0
    freqs = (1.0 / (THETA ** (q / np.float32(N))) / np.float32(TWO_PI)).astype(
        np.float32
    )
    phases = np.arange(T, dtype=np.float32)[:, None] * freqs[None, :]
    ph = (phases % 1.0) * np.float32(TWO_PI)
    cosf = np.cos(ph).astype(np.float32)  # [T, N]
    sinf = np.sin(ph).astype(np.float32)
    QR = np.empty_like(Q)
    Qe, Qo = Q[..., 0::2], Q[..., 1::2]
    ce, se = cosf[:, 0::2], sinf[:, 0::2]
    QR[..., 0::2] = Qe * ce - Qo * se
    QR[..., 1::2] = Qo * ce + Qe * se

    # pair-tile masks [P, 2, 2P]: row 0 = [strict-triu | ones] (G_j0: diag
    # block then the full j1 block), row 1 = [zeros | strict-triu] (G_j1)
    mskT = np.zeros((P, 2, SUP * P), np.float32)
    mskT[:, 0, :P] = np.triu(np.ones((P, P), np.float32), k=1)
    mskT[:, 0, P:] = 1.0
    mskT[:, 1, P:] = np.triu(np.ones((P, P), np.float32), k=1)
    mskT = mskT.astype(f16_np)

    # deinterleave pairs: planes (evens, odds), cast fp16
    Qd = np.stack([QR[..., 0::2], QR[..., 1::2]], axis=2).astype(f16_np)
    # Qd: [B, NH, 2, T, 128]
    # per-(bh, sup) piece, 2KB contiguous per partition:
    #   cols [0:512]    qt: [m, t'] -> Qd[b,h,m, s*256 + t', k=p]
    #   cols [512:1024] qn: [m, ci, k] -> Qd[b,h,m, s*256+ci*128+p, k]
    Qt = (
        Qd.transpose(0, 1, 4, 2, 3)              # [B,NH,k,2,T]
        .reshape(B, NH, P, 2, NSUP, SUP * P)
        .transpose(0, 1, 4, 2, 3, 5)             # [B,NH,NSUP,k,2,256]
        .reshape(B, NH, NSUP, P, 512)
    )
    Qn = (
        Qd.reshape(B, NH, 2, NSUP, SUP, P, P)    # [B,NH,m,s,ci,t',k]
        .transpose(0, 1, 3, 5, 2, 4, 6)          # [B,NH,s,t',m,ci,k]
        .reshape(B, NH, NSUP, P, 512)
    )
    q2 = np.ascontiguousarray(np.concatenate([Qt, Qn], axis=-1))
    # q2: [B, NH, NSUP, P, 1024]

    V16 = V.astype(f16_np)
    # v layout [P, NCH, D]: (p, c, d) = V[c*128+p, d]
    Vp = np.ascontiguousarray(V16.reshape(B, NCH, P, D).transpose(0, 2, 1, 3))
    V8p = Vp.astype(f8_np)

    in_maps = []
    for core in range(NCORES):
        b = core // (NCORES // B)
        hs = (core % (NCORES // B)) * HPC
        in_maps.append(
            {
                "q": np.ascontiguousarray(q2[b, hs : hs + HPC]),
                "v": Vp[b],
                "v8": V8p[b],
                "mskT": mskT,
            }
        )
    return in_maps


def _run(inputs, trace=False, **kw):
    nc = _build()
    in_maps = _host_prep(inputs["Q_raw"], inputs["V_raw"])
    res = run_bass_kernel_spmd(nc, in_maps, list(range(NCORES)), trace=trace, **kw)
    out = np.empty((B, NH, T, D), dtype=np.float32)
    for core in range(NCORES):
        b = core // (NCORES // B)
        hs = (core % (NCORES // B)) * HPC
        # device out: [HPC, P, NCH, D] partition-major -> [HPC, T, D]
        o = res.results[core]["out"].astype(np.float32)
        out[b, hs : hs + HPC] = o.transpose(0, 2, 1, 3).reshape(HPC, T, D)
    return out, res


def kernel(**inputs):
    out, _ = _run(inputs)
    return out


# revision 24
# speedup vs baseline: 1.1324x; 1.0203x over previous
"""Trainium2 Bass kernel for BDH recurrent (chunked linear) attention.

Problem shapes (hardcoded): Q_raw [2,16,2048,256] f32, V_raw [2,2048,1024] f32,
out [2,16,2048,1024] f32.  8 NeuronCores, data+head parallel: each core owns
4 (batch, head) pairs; V is shared across the 4 heads of a core's batch.

Math (reference semantics), per (b,h), chunks of 128:
  QR = rope(Q); KR = QR
  out_c = q_c @ state_{<c} + (q_c q_c^T  * strict_tril) v_c
  state += q_c^T v_c

Design:
  * RoPE is a fixed elementwise map of the input, so it is precomputed on
    the host (in fp32, then cast fp16) in the layouts the matmuls need.
  * Per (bh, superchunk) the transposed q (G lhsT/rhs + inter lhsT) and
    natural q (state-update lhsT) are packed in ONE 256KB DRAM piece with
    2KB-contiguous per-partition lines (1 descriptor per DMA engine), so
    each piece moves at wire speed with minimal latency; the first piece
    gates the first matmul at ~9.5us instead of 12.5us.
  * fp16 for all 16-bit work; fp8(e4m3) DoubleRow PV matmuls: per
    superchunk of SUP=2 chunks, the transposed score blocks G_j are
    evacuated into one [128, 2, 256] fp8 pair tile; the PV for chunk i is
    ONE DoubleRow matmul per D-half contracting 256 rows at 2x rate.
    The pair row of the later chunk j1 is zeroed in its leading 128 cols
    by a gpsimd memset (free engine), the rest by mask-multiplies on DVE.
  * PSUM-resident fp32 state; cast to fp16 SBUF right after each
    superchunk's accumulation group closes, split by m-plane across
    scalar (m0) and vector (m1) so the next superchunk's inter matmuls
    unblock as early as possible.
  * PSUM out evacuation split across scalar/vector by a 5:3 chunk
    pattern to balance the two PSUM-capable engines.
All DRAM layouts are partition-major; the output is written
partition-major and un-permuted on host.
"""

import numpy as np
import ml_dtypes

import concourse.mybir as mybir
import concourse.tile as tile
from concourse import bacc
from concourse.bass import ds
from concourse.bass_utils import run_bass_kernel_spmd

B, NH, T, N, D = 2, 16, 2048, 256, 1024
P = 128          # partition / chunk size
NCH = T // P     # 16 chunks
SUP = 2          # chunks per superchunk
NSUP = NCH // SUP
HPC = 4          # (b,h) pairs per core
NCORES = 8
THETA = 2.0 ** 16
TWO_PI = 2.0 * np.pi

f16 = mybir.dt.float16
f8 = mybir.dt.float8e4
f32 = mybir.dt.float32
f16_np = np.float16
f8_np = ml_dtypes.float8_e4m3  # TRN-style e4m3 (max normal 240)

mult = mybir.AluOpType.mult
DR = mybir.MatmulPerfMode.DoubleRow

# engine assignment knobs.
OUT_EVAC_ENG = ("s", "v")     # by chunk parity
STATE_CAST_ENG = ("s", "s")   # by m-plane


def _copy(nc, c, out, in_):
    if c == "s":
        nc.scalar.copy(out, in_)
    else:
        nc.vector.tensor_copy(out, in_)


def _emit_body(nc, tc, qX, v, v8, mskT, out):
    """Tile program for one core: 4 (b,h) pairs, full scan each."""
    with (
        tc.tile_pool(name="const", bufs=1) as constp,
        tc.tile_pool(name="qpool", bufs=2) as qpool,
        tc.tile_pool(name="work", bufs=4) as work,
        tc.tile_pool(name="outbuf", bufs=2) as outp,
        tc.tile_pool(name="statesb", bufs=2) as statep,
        tc.tile_pool(name="ps_state", bufs=1, space="PSUM") as ps_state,
        # one 4-bank ring shared by the per-chunk out tiles AND the
        # pipelined G tile (gens per sup: c0h0, c0h1, g, c1h0, c1h1).
        # With depth 4, each inter-start's WAR reaches an evacuation
        # that completed >= half a superchunk earlier -- depth 3 cost a
        # ~336ns PE stall per superchunk waiting on the odd-chunk evac.
        tc.tile_pool(name="ps_out", bufs=4, space="PSUM") as ps_out,
    ):
        msk_sb = constp.tile([P, 2, 256], f16)
        v8_sb = constp.tile([P, NCH, D], f8)
        v_sb = constp.tile([P, NCH, D], f16)
        # DMA model: the 16 hardware DMA engines are a SHARED pool that
        # serves all queues roughly in doorbell (issue) order, and bh0's
        # scan demands ~300GB/s (v 6MB + q 2MB + out 4MB over ~40us) --
        # right at the pool's capacity.  So pieces are issued JUST IN
        # TIME, in consumption order: only the critical set up front,
        # everything else from inside the scan loop.  scalar's first
        # issue is delayed ~1.3us by its ACT table load, so it only gets
        # later pieces.
        nc.sync.dma_start(msk_sb[:], mskT[:, :, :])
        nc.sync.dma_start(v8_sb[:, 0:2], v8[:, 0:2, :])
        nc.sync.dma_start(v_sb[:, 0:2], v[:, 0:2, :])

        # HAM warmup: the PE clock sits at 1.2GHz until ~3.4us of
        # sustained matmul activity.  The PE is idle during the DMA
        # prologue anyway, so spend it on zero matmuls into the first
        # out-ring generation (never read) -- real matmuls then start at
        # the full 2.4GHz instead of paying a half-rate ramp.
        wtile = constp.tile([P, 512], f16)
        nc.gpsimd.memset(wtile[:], 0.0)
        warm_ps = ps_out.tile([P, 512], f32, tag="outp", name="warm_ps")
        for _ in range(12):
            nc.tensor.matmul(
                warm_ps[:], wtile[:, ds(0, 128)], wtile[:],
                start=True, stop=True,
            )

        def emit_v_pair(k):
            # v8/v chunks [2k, 2k+2): v8 on scalar, v on sync
            nc.scalar.dma_start(v8_sb[:, 2 * k : 2 * k + 2], v8[:, 2 * k : 2 * k + 2, :])
            nc.sync.dma_start(v_sb[:, 2 * k : 2 * k + 2], v[:, 2 * k : 2 * k + 2, :])

        def q_piece(q_sb, bh, s):
            nc.gpsimd.dma_start(q_sb[:, s, :], qX[bh, s, :, :])

        def bh_prologue(bh, nsup0=2):
            """Allocate the per-bh q tile + DMA its first nsup0 pieces."""
            q_sb = qpool.tile([P, NSUP, 1024], f16, tag="q", name=f"q{bh}")
            for s in range(nsup0):
                q_piece(q_sb, bh, s)
            return q_sb

        # within a sup piece: qt(m, t') at col m*256 + t'; qn(m, ci, k)
        # at col 512 + m*256 + ci*128 + k.
        def emit_G(q_tile, s):
            # Transposed score blocks for superchunk s's two chunks into
            # one PSUM tile: G_j0 at cols 0:256 (diag block + the j1
            # block), G_j1 at 256:384; then the masked fp8 evacuation
            # into a pair tile [p, j', i-col]: row 0 = G_j0 (diag-masked
            # then ones), row 1 = [zeros (gpsimd memset) | G_j1
            # diag-masked].  Called one superchunk AHEAD so the
            # evacuation is off the boundary's critical path.
            g_ps = ps_out.tile([P, 512], f32, tag="outp", name="g_ps")
            nc.tensor.matmul(
                g_ps[:, 0:256], q_tile[:, s, ds(0, 128)],
                q_tile[:, s, ds(0, 256)], start=True, stop=False,
            )
            nc.tensor.matmul(
                g_ps[:, 0:256], q_tile[:, s, ds(256, 128)],
                q_tile[:, s, ds(256, 256)], start=False, stop=True,
            )
            nc.tensor.matmul(
                g_ps[:, 256:384], q_tile[:, s, ds(128, 128)],
                q_tile[:, s, ds(128, 128)], start=True, stop=False,
            )
            nc.tensor.matmul(
                g_ps[:, 256:384], q_tile[:, s, ds(384, 128)],
                q_tile[:, s, ds(384, 128)], start=False, stop=True,
            )
            g2 = work.tile([P, 2, 256], f8, tag="g2", name="g2")
            nc.vector.tensor_tensor(g2[:, 0], g_ps[:, 0:256], msk_sb[:, 0], mult)
            nc.vector.tensor_tensor(g2[:, 1], g_ps[:, 128:384], msk_sb[:, 1], mult)
            return g2

        nxt = bh_prologue(0)
        g2_cur = emit_G(nxt, 0)
        for bh in range(HPC):
            q_sb = nxt
            nxt = None

            state_ps = ps_state.tile([P, 2, D], f32, tag="state")
            out_sbs = [
                outp.tile([P, NCH // 2, D], f16, tag=f"out{h}", name=f"out_sb{h}")
                for h in range(2)
            ]

            def emit_state_chunk(s, ci, i, is_start, is_close):
                # state += qr_c^T v_c (PSUM accumulate).  Each
                # superchunk's accumulation is a CLOSED group; on close,
                # each m-plane is cast fp32->fp16 on its own engine
                # (scalar m0 / vector m1) immediately, so the next
                # superchunk's inter matmuls unblock per-plane.
                sb = (
                    statep.tile([P, 2, D], f16, tag="state_sb", name="state_sb")
                    if is_close else None
                )
                for m in range(2):
                    for h in range(2):
                        dsl = ds(h * 512, 512)
                        nc.tensor.matmul(
                            state_ps[:, m, dsl],
                            q_sb[:, s, ds(512 + m * 256 + ci * 128, 128)],
                            v_sb[:, i, dsl],
                            start=is_start,
                            stop=is_close,
                            skip_group_check=True,
                        )
                    if is_close:
                        _copy(
                            nc, STATE_CAST_ENG[m], sb[:, m, :], state_ps[:, m, :]
                        )
                return sb

            state_sb_next = None
            for s in range(NSUP):
                state_sb = state_sb_next
                j0 = SUP * s
                g2 = g2_cur

                # JIT DMA issue, ~2 superchunks ahead of consumption:
                # own q piece s+2, then the next bh's first pieces; v
                # pairs (resident after bh0) one superchunk ahead.
                if s + 2 < NSUP:
                    q_piece(q_sb, bh, s + 2)
                elif bh < HPC - 1:
                    if s == NSUP - 2:
                        nxt = bh_prologue(bh + 1, nsup0=0)
                    q_piece(nxt, bh + 1, s + 2 - NSUP)
                if bh == 0 and s + 1 < NSUP:
                    emit_v_pair(s + 1)

                # Emit the whole superchunk's state matmuls + the closing
                # casts BEFORE the out-blocks: the casts then precede the
                # out evacuations in scalar's strict FIFO, landing ~2us
                # before the next superchunk's inter matmuls need them
                # (emitting them after the evacs cost ~600ns of PE stall
                # per superchunk).  State after the last superchunk is
                # never read -> skipped.  bh0's sup0 is deferred below so
                # the first PV isn't queued behind it during the DMA
                # prologue.
                if s < NSUP - 1 and not (s == 0 and bh == 0):
                    for ci in range(SUP):
                        sb = emit_state_chunk(
                            s, ci, SUP * s + ci,
                            s == 0 and ci == 0, ci == SUP - 1,
                        )
                    state_sb_next = sb

                for ci in range(SUP):
                    i = SUP * s + ci
                    out_ps = [
                        ps_out.tile([P, 512], f32, tag="outp", name=f"out_ps{h}")
                        for h in range(2)
                    ]
                    if s > 0:
                        # m-outer / h-inner: consecutive matmuls share lhsT
                        for m in range(2):
                            for h in range(2):
                                nc.tensor.matmul(
                                    out_ps[h][:],
                                    q_sb[:, s, ds(m * 256 + ci * 128, 128)],
                                    state_sb[:, m, ds(h * 512, 512)],
                                    start=(m == 0), stop=False,
                                    skip_group_check=True,
                                )
                    # PV: one fp8 DoubleRow matmul per D-half, contracting
                    # both chunks of the superchunk at 2x rate.
                    for h in range(2):
                        nc.tensor.matmul(
                            out_ps[h][:],
                            g2[:, :, ds(ci * P, P)],
                            v8_sb[:, ds(j0, SUP), ds(h * 512, 512)],
                            start=(s == 0), stop=True,
                            perf_mode=DR,
                            skip_group_check=True,
                        )

                    out_sb = out_sbs[i // (NCH // 2)]
                    if i == NCH - 1 and bh == HPC - 1:
                        engs = ("s", "v")  # final chunk: minimize drain latency
                    else:
                        e = OUT_EVAC_ENG[i % len(OUT_EVAC_ENG)]
                        engs = (e, e)
                    for h in range(2):
                        _copy(
                            nc, engs[h],
                            out_sb[:, i % (NCH // 2), ds(h * 512, 512)],
                            out_ps[h][:],
                        )
                    if ci == 0:
                        # pipeline: emit the NEXT superchunk's G + fp8 evac
                        # now (PE runs it after this sup's remaining work;
                        # the DVE evac lands before the next PV needs it)
                        if s + 1 < NSUP:
                            g2_cur = emit_G(q_sb, s + 1)
                        elif bh < HPC - 1:
                            g2_cur = emit_G(nxt, 0)
                    if s == NSUP - 1 and bh == HPC - 1:
                        # drain tail: per-chunk, D-halves on two different
                        # queues so the final transfers ride parallel rings
                        nc.sync.dma_start(
                            out[bh, :, ds(i, 1), ds(0, 512)],
                            out_sbs[i // (NCH // 2)][:, ds(i % (NCH // 2), 1), ds(0, 512)],
                        )
                        nc.scalar.dma_start(
                            out[bh, :, ds(i, 1), ds(512, 512)],
                            out_sbs[i // (NCH // 2)][:, ds(i % (NCH // 2), 1), ds(512, 512)],
                        )
                    elif ci == SUP - 1:
                        for cc in range(SUP):
                            nc.sync.dma_start(
                                out[bh, :, ds(j0 + cc, 1), :],
                                out_sbs[j0 // (NCH // 2)][
                                    :, ds((j0 + cc) % (NCH // 2), 1)
                                ],
                            )

                if s == 0 and bh == 0:
                    # bh0 sup0's state is deferred past its PV so the PV
                    # starts ASAP after the prologue.
                    for ci2 in range(SUP):
                        sb = emit_state_chunk(
                            0, ci2, ci2, ci2 == 0, ci2 == SUP - 1
                        )
                    state_sb_next = sb


_BUILT = {}


def _build():
    if "nc" in _BUILT:
        return _BUILT["nc"]
    nc = bacc.Bacc(
        "TRN2", target_bir_lowering=False, debug=False,
        enable_asserts=True, num_devices=NCORES,
    )
    qX = nc.dram_tensor("q", [HPC, NSUP, P, 1024], f16, kind="ExternalInput")
    v = nc.dram_tensor("v", [P, NCH, D], f16, kind="ExternalInput")
    v8 = nc.dram_tensor("v8", [P, NCH, D], f8, kind="ExternalInput")
    mskT = nc.dram_tensor("mskT", [P, 2, SUP * P], f16, kind="ExternalInput")
    out = nc.dram_tensor("out", [HPC, P, NCH, D], f16, kind="ExternalOutput")
    with tile.TileContext(nc) as tc:
        _emit_body(nc, tc, qX, v, v8, mskT, out)
    nc.compile()
    _BUILT["nc"] = nc
    return nc


def _host_prep(Q_raw, V_raw):
    """Shard + precompute device inputs (fp16/fp8, partition-major),
    including the RoPE rotation (an input-only elementwise transform),
    computed in fp32 exactly as the reference does."""
    Q = np.asarray(Q_raw, dtype=np.float32)
    V = np.asarray(V_raw, dtype=np.float32)

    t = np.arange(N, dtype=np.float32)
    q = np.floor(t / 2.0) * 2.0
    freqs = (1.0 / (THETA ** (q / np.float32(N))) / np.float32(TWO_PI)).astype(
        np.float32
    )
    phases = np.arange(T, dtype=np.float32)[:, None] * freqs[None, :]
    ph = (phases % 1.0) * np.float32(TWO_PI)
    cosf = np.cos(ph).astype(np.float32)  # [T, N]
    sinf = np.sin(ph).astype(np.float32)
    QR = np.empty_like(Q)
    Qe, Qo = Q[..., 0::2], Q[..., 1::2]
    ce, se = cosf[:, 0::2], sinf[:, 0::2]
    QR[..., 0::2] = Qe * ce - Qo * se
    QR[..., 1::2] = Qo * ce + Qe * se

    # pair-tile masks [P, 2, 2P]: row 0 = [strict-triu | ones] (G_j0: diag
    # block then the full j1 block), row 1 = [zeros | strict-triu] (G_j1)
    mskT = np.zeros((P, 2, SUP * P), np.float32)
    mskT[:, 0, :P] = np.triu(np.ones((P, P), np.float32), k=1)
    mskT[:, 0, P:] = 1.0
    mskT[:, 1, P:] = np.triu(np.ones((P, P), np.float32), k=1)
    mskT = mskT.astype(f16_np)

    # deinterleave pairs: planes (evens, odds), cast fp16
    Qd = np.stack([QR[..., 0::2], QR[..., 1::2]], axis=2).astype(f16_np)
    # Qd: [B, NH, 2, T, 128]
    # per-(bh, sup) piece, 2KB contiguous per partition:
    #   cols [0:512]    qt: [m, t'] -> Qd[b,h,m, s*256 + t', k=p]
    #   cols [512:1024] qn: [m, ci, k] -> Qd[b,h,m, s*256+ci*128+p, k]
    Qt = (
        Qd.transpose(0, 1, 4, 2, 3)              # [B,NH,k,2,T]
        .reshape(B, NH, P, 2, NSUP, SUP * P)
        .transpose(0, 1, 4, 2, 3, 5)             # [B,NH,NSUP,k,2,256]
        .reshape(B, NH, NSUP, P, 512)
    )
    Qn = (
        Qd.reshape(B, NH, 2, NSUP, SUP, P, P)    # [B,NH,m,s,ci,t',k]
        .transpose(0, 1, 3, 5, 2, 4, 6)          # [B,NH,s,t',m,ci,k]
        .reshape(B, NH, NSUP, P, 512)
    )
    q2 = np.ascontiguousarray(np.concatenate([Qt, Qn], axis=-1))
    # q2: [B, NH, NSUP, P, 1024]

    V16 = V.astype(f16_np)
    # v layout [P, NCH, D]: (p, c, d) = V[c*128+p, d]
    Vp = np.ascontiguousarray(V16.reshape(B, NCH, P, D).transpose(0, 2, 1, 3))
    V8p = Vp.astype(f8_np)

    in_maps = []
    for core in range(NCORES):
        b = core // (NCORES // B)
        hs = (core % (NCORES // B)) * HPC
        in_maps.append(
            {
                "q": np.ascontiguousarray(q2[b, hs : hs + HPC]),
                "v": Vp[b],
                "v8": V8p[b],
                "mskT": mskT,
            }
        )
    return in_maps


def _run(inputs, trace=False, **kw):
    nc = _build()
    in_maps = _host_prep(inputs["Q_raw"], inputs["V_raw"])
    res = run_bass_kernel_spmd(nc, in_maps, list(range(NCORES)), trace=trace, **kw)
    out = np.empty((B, NH, T, D), dtype=np.float32)
    for core in range(NCORES):
        b = core // (NCORES // B)
        hs = (core % (NCORES // B)) * HPC
        # device out: [HPC, P, NCH, D] partition-major -> [HPC, T, D]
        o = res.results[core]["out"].astype(np.float32)
        out[b, hs : hs + HPC] = o.transpose(0, 2, 1, 3).reshape(HPC, T, D)
    return out, res


def kernel(**inputs):
    out, _ = _run(inputs)
    return out


# revision 28
# speedup vs baseline: 1.1448x; 1.0109x over previous
"""Trainium2 Bass kernel for BDH recurrent (chunked linear) attention.

Problem shapes (hardcoded): Q_raw [2,16,2048,256] f32, V_raw [2,2048,1024] f32,
out [2,16,2048,1024] f32.  8 NeuronCores, data+head parallel: each core owns
4 (batch, head) pairs; V is shared across the 4 heads of a core's batch.

Math (reference semantics), per (b,h), chunks of 128:
  QR = rope(Q); KR = QR
  out_c = q_c @ state_{<c} + (q_c q_c^T  * strict_tril) v_c
  state += q_c^T v_c

Design:
  * RoPE is a fixed elementwise map of the input, so it is precomputed on
    the host (in fp32, then cast fp16) in the layouts the matmuls need.
  * Per (bh, superchunk) the transposed q (G lhsT/rhs + inter lhsT) and
    natural q (state-update lhsT) are packed in ONE 256KB DRAM piece with
    2KB-contiguous per-partition lines (1 descriptor per DMA engine), so
    each piece moves at wire speed with minimal latency; the first piece
    gates the first matmul at ~9.5us instead of 12.5us.
  * fp16 for all 16-bit work; fp8(e4m3) DoubleRow PV matmuls: per
    superchunk of SUP=2 chunks, the transposed score blocks G_j are
    evacuated into one [128, 2, 256] fp8 pair tile; the PV for chunk i is
    ONE DoubleRow matmul per D-half contracting 256 rows at 2x rate.
    The pair row of the later chunk j1 is zeroed in its leading 128 cols
    by a gpsimd memset (free engine), the rest by mask-multiplies on DVE.
  * PSUM-resident fp32 state; cast to fp16 SBUF right after each
    superchunk's accumulation group closes, split by m-plane across
    scalar (m0) and vector (m1) so the next superchunk's inter matmuls
    unblock as early as possible.
  * PSUM out evacuation split across scalar/vector by a 5:3 chunk
    pattern to balance the two PSUM-capable engines.
All DRAM layouts are partition-major; the output is written
partition-major and un-permuted on host.
"""

import numpy as np
import ml_dtypes

import concourse.mybir as mybir
import concourse.tile as tile
from concourse import bacc
from concourse.bass import ds
from concourse.bass_utils import run_bass_kernel_spmd

B, NH, T, N, D = 2, 16, 2048, 256, 1024
P = 128          # partition / chunk size
NCH = T // P     # 16 chunks
SUP = 2          # chunks per superchunk
NSUP = NCH // SUP
HPC = 4          # (b,h) pairs per core
NCORES = 8
THETA = 2.0 ** 16
TWO_PI = 2.0 * np.pi

f16 = mybir.dt.float16
f8 = mybir.dt.float8e4
f32 = mybir.dt.float32
f16_np = np.float16
f8_np = ml_dtypes.float8_e4m3  # TRN-style e4m3 (max normal 240)

mult = mybir.AluOpType.mult
DR = mybir.MatmulPerfMode.DoubleRow

# engine assignment knobs.
OUT_EVAC_ENG = ("s", "v")     # by chunk parity
STATE_CAST_ENG = ("s", "s")   # by m-plane


def _copy(nc, c, out, in_):
    if c == "s":
        nc.scalar.copy(out, in_)
    else:
        nc.vector.tensor_copy(out, in_)


def _emit_body(nc, tc, qX, v, v8, mskT, out):
    """Tile program for one core: 4 (b,h) pairs, full scan each."""
    with (
        tc.tile_pool(name="const", bufs=1) as constp,
        tc.tile_pool(name="qpool", bufs=2) as qpool,
        tc.tile_pool(name="work", bufs=4) as work,
        tc.tile_pool(name="outbuf", bufs=2) as outp,
        tc.tile_pool(name="statesb", bufs=2) as statep,
        tc.tile_pool(name="ps_state", bufs=1, space="PSUM") as ps_state,
        # one 4-bank ring shared by the per-chunk out tiles AND the
        # pipelined G tile (gens per sup: c0h0, c0h1, g, c1h0, c1h1).
        # With depth 4, each inter-start's WAR reaches an evacuation
        # that completed >= half a superchunk earlier -- depth 3 cost a
        # ~336ns PE stall per superchunk waiting on the odd-chunk evac.
        tc.tile_pool(name="ps_out", bufs=4, space="PSUM") as ps_out,
    ):
        msk_sb = constp.tile([P, 2, 256], f16)
        v8_sb = constp.tile([P, NCH, D], f8)
        v_sb = constp.tile([P, NCH, D], f16)
        # DMA model: the 16 hardware DMA engines are a SHARED pool that
        # serves all queues roughly in doorbell (issue) order, and bh0's
        # scan demands ~300GB/s (v 6MB + q 2MB + out 4MB over ~40us) --
        # right at the pool's capacity.  So pieces are issued JUST IN
        # TIME, in consumption order: only the critical set up front,
        # everything else from inside the scan loop.  scalar's first
        # issue is delayed ~1.3us by its ACT table load, so it only gets
        # later pieces.
        # NB: the scheduler HOISTS all dependency-free dma_start issues
        # to the front of each engine's stream, so only per-queue FIFO
        # order is controllable.  Give each of the three queues a head
        # of critical pieces in consumption order; the shared 16-engine
        # pool multiplexes the queue heads.
        wtile = constp.tile([P, 512], f16)
        nc.gpsimd.memset(wtile[:], 0.0)

        q_sb0 = qpool.tile([P, NSUP, 1024], f16, tag="q", name="q_sb0")
        nc.sync.dma_start(q_sb0[:, 0, :], qX[0, 0, :, :])
        nc.sync.dma_start(msk_sb[:], mskT[:, :, :])
        nc.sync.dma_start(v8_sb[:, 0:2], v8[:, 0:2, :])
        nc.gpsimd.dma_start(v_sb[:, 0:2], v[:, 0:2, :])
        nc.gpsimd.dma_start(q_sb0[:, 1, :], qX[0, 1, :, :])

        def warm(n):
            # HAM warmup / keep-warm: the PE clock sits at 1.2GHz until
            # ~3.4us of sustained matmul activity, and re-throttles
            # after ~3.4us idle.  The PE is starved during the DMA
            # prologue anyway, so fill the holes with zero matmuls into
            # dead out-ring generations (never read): all real matmuls
            # then run at the full 2.4GHz.
            w_ps = ps_out.tile([P, 512], f32, tag="outp", name="warm_ps")
            for _ in range(n):
                nc.tensor.matmul(
                    w_ps[:], wtile[:, ds(0, 128)], wtile[:],
                    start=True, stop=True,
                )

        warm(12)

        def emit_v_pair(k):
            # v8/v chunks [2k, 2k+2): v8 on scalar, v on sync
            nc.scalar.dma_start(v8_sb[:, 2 * k : 2 * k + 2], v8[:, 2 * k : 2 * k + 2, :])
            nc.sync.dma_start(v_sb[:, 2 * k : 2 * k + 2], v[:, 2 * k : 2 * k + 2, :])

        def q_piece(q_sb, bh, s):
            nc.gpsimd.dma_start(q_sb[:, s, :], qX[bh, s, :, :])

        def bh_prologue(bh):
            """Allocate the per-bh q tile (pieces DMA'd from the loop)."""
            return qpool.tile([P, NSUP, 1024], f16, tag="q", name=f"q{bh}")

        # within a sup piece: qt(m, t') at col m*256 + t'; qn(m, ci, k)
        # at col 512 + m*256 + ci*128 + k.
        def emit_G(q_tile, s):
            # Transposed score blocks for superchunk s's two chunks into
            # one PSUM tile: G_j0 at cols 0:256 (diag block + the j1
            # block), G_j1 at 256:384; then the masked fp8 evacuation
            # into a pair tile [p, j', i-col]: row 0 = G_j0 (diag-masked
            # then ones), row 1 = [zeros (gpsimd memset) | G_j1
            # diag-masked].  Called one superchunk AHEAD so the
            # evacuation is off the boundary's critical path.
            g_ps = ps_out.tile([P, 512], f32, tag="outp", name="g_ps")
            nc.tensor.matmul(
                g_ps[:, 0:256], q_tile[:, s, ds(0, 128)],
                q_tile[:, s, ds(0, 256)], start=True, stop=False,
            )
            nc.tensor.matmul(
                g_ps[:, 0:256], q_tile[:, s, ds(256, 128)],
                q_tile[:, s, ds(256, 256)], start=False, stop=True,
            )
            nc.tensor.matmul(
                g_ps[:, 256:384], q_tile[:, s, ds(128, 128)],
                q_tile[:, s, ds(128, 128)], start=True, stop=False,
            )
            nc.tensor.matmul(
                g_ps[:, 256:384], q_tile[:, s, ds(384, 128)],
                q_tile[:, s, ds(384, 128)], start=False, stop=True,
            )
            g2 = work.tile([P, 2, 256], f8, tag="g2", name="g2")
            nc.vector.tensor_tensor(g2[:, 0], g_ps[:, 0:256], msk_sb[:, 0], mult)
            nc.vector.tensor_tensor(g2[:, 1], g_ps[:, 128:384], msk_sb[:, 1], mult)
            return g2

        nxt = q_sb0
        g2_cur = emit_G(nxt, 0)
        warm(6)
        for bh in range(HPC):
            q_sb = nxt
            nxt = None

            state_ps = ps_state.tile([P, 2, D], f32, tag="state")
            out_sbs = [
                outp.tile([P, NCH // 2, D], f16, tag=f"out{h}", name=f"out_sb{h}")
                for h in range(2)
            ]

            def emit_state_chunk(s, ci, i, is_start, is_close):
                # state += qr_c^T v_c (PSUM accumulate).  Each
                # superchunk's accumulation is a CLOSED group; on close,
                # each m-plane is cast fp32->fp16 on its own engine
                # (scalar m0 / vector m1) immediately, so the next
                # superchunk's inter matmuls unblock per-plane.
                sb = (
                    statep.tile([P, 2, D], f16, tag="state_sb", name="state_sb")
                    if is_close else None
                )
                for m in range(2):
                    for h in range(2):
                        dsl = ds(h * 512, 512)
                        nc.tensor.matmul(
                            state_ps[:, m, dsl],
                            q_sb[:, s, ds(512 + m * 256 + ci * 128, 128)],
                            v_sb[:, i, dsl],
                            start=is_start,
                            stop=is_close,
                            skip_group_check=True,
                        )
                    if is_close:
                        _copy(
                            nc, STATE_CAST_ENG[m], sb[:, m, :], state_ps[:, m, :]
                        )
                return sb

            state_sb_next = None
            for s in range(NSUP):
                state_sb = state_sb_next
                j0 = SUP * s
                g2 = g2_cur

                # JIT DMA issue, ~2 superchunks ahead of consumption:
                # own q piece s+2, then the next bh's first pieces; v
                # pairs (resident after bh0) one superchunk ahead.
                if s + 2 < NSUP:
                    q_piece(q_sb, bh, s + 2)
                elif bh < HPC - 1:
                    if s == NSUP - 2:
                        nxt = bh_prologue(bh + 1)
                    q_piece(nxt, bh + 1, s + 2 - NSUP)
                if bh == 0 and s + 1 < NSUP:
                    emit_v_pair(s + 1)

                # Emit the whole superchunk's state matmuls + the closing
                # casts BEFORE the out-blocks: the casts then precede the
                # out evacuations in scalar's strict FIFO, landing ~2us
                # before the next superchunk's inter matmuls need them
                # (emitting them after the evacs cost ~600ns of PE stall
                # per superchunk).  State after the last superchunk is
                # never read -> skipped.  bh0's sup0 is deferred below so
                # the first PV isn't queued behind it during the DMA
                # prologue.
                if s < NSUP - 1 and not (s == 0 and bh == 0):
                    for ci in range(SUP):
                        sb = emit_state_chunk(
                            s, ci, SUP * s + ci,
                            s == 0 and ci == 0, ci == SUP - 1,
                        )
                    state_sb_next = sb

                for ci in range(SUP):
                    i = SUP * s + ci
                    out_ps = [
                        ps_out.tile([P, 512], f32, tag="outp", name=f"out_ps{h}")
                        for h in range(2)
                    ]
                    if s > 0:
                        # m-outer / h-inner: consecutive matmuls share lhsT
                        for m in range(2):
                            for h in range(2):
                                nc.tensor.matmul(
                                    out_ps[h][:],
                                    q_sb[:, s, ds(m * 256 + ci * 128, 128)],
                                    state_sb[:, m, ds(h * 512, 512)],
                                    start=(m == 0), stop=False,
                                    skip_group_check=True,
                                )
                    # PV: one fp8 DoubleRow matmul per D-half, contracting
                    # both chunks of the superchunk at 2x rate.
                    for h in range(2):
                        nc.tensor.matmul(
                            out_ps[h][:],
                            g2[:, :, ds(ci * P, P)],
                            v8_sb[:, ds(j0, SUP), ds(h * 512, 512)],
                            start=(s == 0), stop=True,
                            perf_mode=DR,
                            skip_group_check=True,
                        )

                    out_sb = out_sbs[i // (NCH // 2)]
                    if i == NCH - 1 and bh == HPC - 1:
                        engs = ("s", "v")  # final chunk: minimize drain latency
                    else:
                        e = OUT_EVAC_ENG[i % len(OUT_EVAC_ENG)]
                        engs = (e, e)
                    for h in range(2):
                        _copy(
                            nc, engs[h],
                            out_sb[:, i % (NCH // 2), ds(h * 512, 512)],
                            out_ps[h][:],
                        )
                    if ci == 0:
                        # pipeline: emit the NEXT superchunk's G + fp8 evac
                        # now (PE runs it after this sup's remaining work;
                        # the DVE evac lands before the next PV needs it)
                        if s + 1 < NSUP:
                            g2_cur = emit_G(q_sb, s + 1)
                        elif bh < HPC - 1:
                            g2_cur = emit_G(nxt, 0)
                        if bh == 0 and s == 0:
                            # keep the PE/HAM warm across the remaining
                            # prologue DMA starvation window
                            warm(6)
                    if s == NSUP - 1 and bh == HPC - 1:
                        # drain tail: per-chunk, D-halves on two different
                        # queues so the final transfers ride parallel rings
                        nc.sync.dma_start(
                            out[bh, :, ds(i, 1), ds(0, 512)],
                            out_sbs[i // (NCH // 2)][:, ds(i % (NCH // 2), 1), ds(0, 512)],
                        )
                        nc.scalar.dma_start(
                            out[bh, :, ds(i, 1), ds(512, 512)],
                            out_sbs[i // (NCH // 2)][:, ds(i % (NCH // 2), 1), ds(512, 512)],
                        )
                    elif ci == SUP - 1:
                        for cc in range(SUP):
                            nc.sync.dma_start(
                                out[bh, :, ds(j0 + cc, 1), :],
                                out_sbs[j0 // (NCH // 2)][
                                    :, ds((j0 + cc) % (NCH // 2), 1)
                                ],
                            )

                if s == 0 and bh == 0:
                    # bh0 sup0's state is deferred past its PV so the PV
                    # starts ASAP after the prologue.
                    for ci2 in range(SUP):
                        sb = emit_state_chunk(
                            0, ci2, ci2, ci2 == 0, ci2 == SUP - 1
                        )
                    state_sb_next = sb


_BUILT = {}


def _build():
    if "nc" in _BUILT:
        return _BUILT["nc"]
    nc = bacc.Bacc(
        "TRN2", target_bir_lowering=False, debug=False,
        enable_asserts=True, num_devices=NCORES,
    )
    qX = nc.dram_tensor("q", [HPC, NSUP, P, 1024], f16, kind="ExternalInput")
    v = nc.dram_tensor("v", [P, NCH, D], f16, kind="ExternalInput")
    v8 = nc.dram_tensor("v8", [P, NCH, D], f8, kind="ExternalInput")
    mskT = nc.dram_tensor("mskT", [P, 2, SUP * P], f16, kind="ExternalInput")
    out = nc.dram_tensor("out", [HPC, P, NCH, D], f16, kind="ExternalOutput")
    with tile.TileContext(nc) as tc:
        _emit_body(nc, tc, qX, v, v8, mskT, out)
    nc.compile()
    _BUILT["nc"] = nc
    return nc


def _host_prep(Q_raw, V_raw):
    """Shard + precompute device inputs (fp16/fp8, partition-major),
    including the RoPE rotation (an input-only elementwise transform),
    computed in fp32 exactly as the reference does."""
    Q = np.asarray(Q_raw, dtype=np.float32)
    V = np.asarray(V_raw, dtype=np.float32)

    t = np.arange(N, dtype=np.float32)
    q = np.floor(t / 2.0) * 2.0
    freqs = (1.0 / (THETA ** (q / np.float32(N))) / np.float32(TWO_PI)).astype(
        np.float32
    )
    phases = np.arange(T, dtype=np.float32)[:, None] * freqs[None, :]
    ph = (phases % 1.0) * np.float32(TWO_PI)
    cosf = np.cos(ph).astype(np.float32)  # [T, N]
    sinf = np.sin(ph).astype(np.float32)
    QR = np.empty_like(Q)
    Qe, Qo = Q[..., 0::2], Q[..., 1::2]
    ce, se = cosf[:, 0::2], sinf[:, 0::2]
    QR[..., 0::2] = Qe * ce - Qo * se
    QR[..., 1::2] = Qo * ce + Qe * se

    # pair-tile masks [P, 2, 2P]: row 0 = [strict-triu | ones] (G_j0: diag
    # block then the full j1 block), row 1 = [zeros | strict-triu] (G_j1)
    mskT = np.zeros((P, 2, SUP * P), np.float32)
    mskT[:, 0, :P] = np.triu(np.ones((P, P), np.float32), k=1)
    mskT[:, 0, P:] = 1.0
    mskT[:, 1, P:] = np.triu(np.ones((P, P), np.float32), k=1)
    mskT = mskT.astype(f16_np)

    # deinterleave pairs: planes (evens, odds), cast fp16
    Qd = np.stack([QR[..., 0::2], QR[..., 1::2]], axis=2).astype(f16_np)
    # Qd: [B, NH, 2, T, 128]
    # per-(bh, sup) piece, 2KB contiguous per partition:
    #   cols [0:512]    qt: [m, t'] -> Qd[b,h,m, s*256 + t', k=p]
    #   cols [512:1024] qn: [m, ci, k] -> Qd[b,h,m, s*256+ci*128+p, k]
    Qt = (
        Qd.transpose(0, 1, 4, 2, 3)              # [B,NH,k,2,T]
        .reshape(B, NH, P, 2, NSUP, SUP * P)
        .transpose(0, 1, 4, 2, 3, 5)             # [B,NH,NSUP,k,2,256]
        .reshape(B, NH, NSUP, P, 512)
    )
    Qn = (
        Qd.reshape(B, NH, 2, NSUP, SUP, P, P)    # [B,NH,m,s,ci,t',k]
        .transpose(0, 1, 3, 5, 2, 4, 6)          # [B,NH,s,t',m,ci,k]
        .reshape(B, NH, NSUP, P, 512)
    )
    q2 = np.ascontiguousarray(np.concatenate([Qt, Qn], axis=-1))
    # q2: [B, NH, NSUP, P, 1024]

    V16 = V.astype(f16_np)
    # v layout [P, NCH, D]: (p, c, d) = V[c*128+p, d]
    Vp = np.ascontiguousarray(V16.reshape(B, NCH, P, D).transpose(0, 2, 1, 3))
    V8p = Vp.astype(f8_np)

    in_maps = []
    for core in range(NCORES):
        b = core // (NCORES // B)
        hs = (core % (NCORES // B)) * HPC
        in_maps.append(
            {
                "q": np.ascontiguousarray(q2[b, hs : hs + HPC]),
                "v": Vp[b],
                "v8": V8p[b],
                "mskT": mskT,
            }
        )
    return in_maps


def _run(inputs, trace=False, **kw):
    nc = _build()
    in_maps = _host_prep(inputs["Q_raw"], inputs["V_raw"])
    res = run_bass_kernel_spmd(nc, in_maps, list(range(NCORES)), trace=trace, **kw)
    out = np.empty((B, NH, T, D), dtype=np.float32)
    for core in range(NCORES):
        b = core // (NCORES // B)
        hs = (core % (NCORES // B)) * HPC
        # device out: [HPC, P, NCH, D] partition-major -> [HPC, T, D]
        o = res.results[core]["out"].astype(np.float32)
        out[b, hs : hs + HPC] = o.transpose(0, 2, 1, 3).reshape(HPC, T, D)
    return out, res


def kernel(**inputs):
    out, _ = _run(inputs)
    return out
